# revision 1
# baseline (speedup 1.0000x reference)
"""Distributed Bass kernel for nn_Interaction_GraphConvolution.

Math (reference):
    x  = node_features @ linear_w.T + linear_b          [N, IN_F]
    wf = x @ weight                                     [N, C]
    G  = mask_father[:,0,:].T @ adjacency               [N, N]
    P  = G * mask_hadamard[:,0,:].T                     [N, N]
    out[c, j] = wf[j,c] * (P @ wf)[j,c] / neighbor_count[c]^2

Sharding: output columns j (node dim) split across 8 cores, 512 each.
Two SPMD launches:
  NEFF-1: core m computes wf rows J_m (512 rows). Host gathers full wf.
  NEFF-2: core m computes G^T/P^T columns J_m and out[:, J_m].
Dtypes: adjacency-side matmuls in bf16 (inputs are small ints - exact);
wf-side matmuls in float32r (~1.5e-4 rel err at full PE rate).
"""

import os
import sys

sys.path.insert(0, "/opt/trn_rl_repo")

import numpy as np
import ml_dtypes

from concourse import bass, bacc, mybir, tile
from concourse.bass_utils import run_bass_kernel_spmd
from concourse.masks import make_identity

F32 = mybir.dt.float32
F32R = mybir.dt.float32r
BF16 = mybir.dt.bfloat16

N = 4096       # nodes (== out channels C)
F_RAW = 512    # raw feature dim
IN_F = 1024    # hidden dim
C = 4096       # out channels
M = 8          # cores
JB = N // M    # 512 output columns per core

LAST_EXEC = {}
LAST_RESULTS = {}


def _build_neff1():
    """Per core: wf_rows[J_m] = (nf[J_m] @ lw.T + b) @ W, via transposed tiles.

    Inputs (per core): lwT [F_RAW, IN_F] f32r, nfT [F_RAW, JB] f32r,
    bias [128, IN_F//128] f32, w [IN_F, C] f32r.
    Output: wf_rows [JB, C] f32.
    """
    nc = bacc.Bacc()
    lwT_d = nc.dram_tensor("lwT", [F_RAW, IN_F], F32R, kind="ExternalInput")
    nfT_d = nc.dram_tensor("nfT", [F_RAW, JB], F32R, kind="ExternalInput")
    b_d = nc.dram_tensor("bias", [128, IN_F // 128], F32, kind="ExternalInput")
    w_d = nc.dram_tensor("w", [IN_F, C], F32R, kind="ExternalInput")
    wf_d = nc.dram_tensor("wf_rows", [JB, C], F32, kind="ExternalOutput")

    NFB = IN_F // 128   # 8 f-blocks
    NRB = F_RAW // 128  # 4 r-blocks
    NJB = JB // 128     # 4 j-blocks
    NCC = C // 512      # 8 c-chunks

    with tile.TileContext(nc) as tc:
        with tc.tile_pool(name="const", bufs=1) as constp:
            lwT_t = constp.tile([128, NRB * IN_F], F32R)
            for rb in range(NRB):
                nc.sync.dma_start(
                    lwT_t[:, rb * IN_F:(rb + 1) * IN_F],
                    lwT_d[rb * 128:(rb + 1) * 128, :])
            nfT_t = constp.tile([128, NRB * JB], F32R)
            for rb in range(NRB):
                nc.sync.dma_start(
                    nfT_t[:, rb * JB:(rb + 1) * JB],
                    nfT_d[rb * 128:(rb + 1) * 128, :])
            b_t = constp.tile([128, NFB], F32)
            nc.sync.dma_start(b_t[:], b_d[:])
            w_t = constp.tile([128, NFB * C], F32R)
            for fb in range(NFB):
                nc.sync.dma_start(
                    w_t[:, fb * C:(fb + 1) * C],
                    w_d[fb * 128:(fb + 1) * 128, :])
            xt_t = constp.tile([128, NFB * JB], F32R)

            # phase X: xT[f, j] = lw @ nf[J_m].T + b
            with tc.tile_pool(name="psx", bufs=2, space=bass.MemorySpace.PSUM) as psxp:
                for fb in range(NFB):
                    psx = psxp.tile([128, JB], F32, tag="psx")
                    for rb in range(NRB):
                        nc.tensor.matmul(
                            psx[:],
                            lwT_t[:, rb * IN_F + fb * 128: rb * IN_F + (fb + 1) * 128],
                            nfT_t[:, rb * JB:(rb + 1) * JB],
                            start=(rb == 0), stop=(rb == NRB - 1))
                    nc.scalar.activation(
                        xt_t[:, fb * JB:(fb + 1) * JB], psx[:],
                        mybir.ActivationFunctionType.Identity,
                        bias=b_t[:, fb:fb + 1], scale=1.0)

            # phase W: wf[J_m] = xT.T @ W
            with tc.tile_pool(name="psw", bufs=8, space=bass.MemorySpace.PSUM) as pswp, \
                 tc.tile_pool(name="io1", bufs=3) as iop:
                for jb in range(NJB):
                    for cc in range(NCC):
                        pw = pswp.tile([128, 512], F32, tag="pw")
                        for fb in range(NFB):
                            nc.tensor.matmul(
                                pw[:],
                                xt_t[:, fb * JB + jb * 128: fb * JB + (jb + 1) * 128],
                                w_t[:, fb * C + cc * 512: fb * C + (cc + 1) * 512],
                                start=(fb == 0), stop=(fb == NFB - 1))
                        o_sb = iop.tile([128, 512], F32, tag="o_sb")
                        nc.vector.tensor_copy(o_sb[:], pw[:])
                        nc.sync.dma_start(
                            wf_d[jb * 128:(jb + 1) * 128, cc * 512:(cc + 1) * 512],
                            o_sb[:])
    nc.finalize()
    return nc


def _build_neff2():
    """Per core: G^T/P^T for columns J_m, then out[:, J_m].

    Inputs: a [N, N] bf16 (adjacency), ao [N, JB] bf16 (mask_father cols),
    s [N, JB] bf16 (mask_hadamard cols), wfd [N, C] f32r (full wf),
    wfs [JB, C] f32 (wf rows J_m, pre-scaled by nothing - raw),
    inv2 [128, N//128] f32 (1/neighbor_count^2 tiled).
    Output: outc [C, JB] f32  (= output[:, J_m]).
    """
    nc = bacc.Bacc()
    a_d = nc.dram_tensor("a", [N, N], BF16, kind="ExternalInput")
    ao_d = nc.dram_tensor("ao", [N, JB], BF16, kind="ExternalInput")
    s_d = nc.dram_tensor("s", [N, JB], BF16, kind="ExternalInput")
    wf_d = nc.dram_tensor("wfd", [N, C], F32R, kind="ExternalInput")
    wr_d = nc.dram_tensor("wfs", [JB, C], F32, kind="ExternalInput")
    i2_d = nc.dram_tensor("inv2", [128, N // 128], F32, kind="ExternalInput")
    out_d = nc.dram_tensor("outc", [C, JB], F32, kind="ExternalOutput")

    NKB = N // 128    # 32 k-blocks
    NIB = N // 128    # 32 i-blocks
    NCB = C // 128    # 32 c-blocks
    NJB = JB // 128   # 4 j-blocks

    with tile.TileContext(nc) as tc:
        with tc.tile_pool(name="const", bufs=1) as constp:
            ident = constp.tile([128, 128], F32)
            make_identity(nc, ident[:])
            i2_t = constp.tile([128, N // 128], F32)
            nc.sync.dma_start(i2_t[:], i2_d[:])
            aot = constp.tile([128, NKB * JB], BF16)
            for kb in range(NKB):
                nc.sync.dma_start(
                    aot[:, kb * JB:(kb + 1) * JB],
                    ao_d[kb * 128:(kb + 1) * 128, :])
            pt_t = constp.tile([128, NIB * JB], F32R)

            # phase G: PT[i, j] = (A^T @ Ao) * S  for j in J_m
            with tc.tile_pool(name="psg", bufs=8, space=bass.MemorySpace.PSUM) as psgp, \
                 tc.tile_pool(name="ioa", bufs=3) as ioa, \
                 tc.tile_pool(name="ios", bufs=2) as ios:
                for isup in range(NIB // 8):
                    psg = [psgp.tile([128, JB], F32, tag="psg", name=f"psg{_i}") for _i in range(8)]
                    for kb in range(NKB):
                        a_t = ioa.tile([128, 1024], BF16, tag="a_t")
                        nc.sync.dma_start(
                            a_t[:],
                            a_d[kb * 128:(kb + 1) * 128,
                                isup * 1024:(isup + 1) * 1024])
                        for ib8 in range(8):
                            nc.tensor.matmul(
                                psg[ib8][:],
                                a_t[:, ib8 * 128:(ib8 + 1) * 128],
                                aot[:, kb * JB:(kb + 1) * JB],
                                start=(kb == 0), stop=(kb == NKB - 1))
                    for ib8 in range(8):
                        ib = isup * 8 + ib8
                        s_t = ios.tile([128, JB], BF16, tag="s_t")
                        nc.sync.dma_start(s_t[:], s_d[ib * 128:(ib + 1) * 128, :])
                        nc.vector.tensor_mul(
                            pt_t[:, ib * JB:(ib + 1) * JB], psg[ib8][:], s_t[:])

            # phase O: out[c, j] = (wf^T @ PT) * wf^T * inv2
            with tc.tile_pool(name="pso", bufs=4, space=bass.MemorySpace.PSUM) as psop, \
                 tc.tile_pool(name="pst", bufs=2, space=bass.MemorySpace.PSUM) as pstp, \
                 tc.tile_pool(name="iow", bufs=4) as iow, \
                 tc.tile_pool(name="ior", bufs=4) as ior, \
                 tc.tile_pool(name="ioo", bufs=3) as ioo:
                for csup in range(NCB // 4):
                    pso = [psop.tile([128, JB], F32, tag="pso", name=f"pso{_i}") for _i in range(4)]
                    for ib in range(NIB):
                        wf_t = iow.tile([128, 512], F32R, tag="wf_t")
                        nc.sync.dma_start(
                            wf_t[:],
                            wf_d[ib * 128:(ib + 1) * 128,
                                 csup * 512:(csup + 1) * 512])
                        for cb4 in range(4):
                            nc.tensor.matmul(
                                pso[cb4][:],
                                wf_t[:, cb4 * 128:(cb4 + 1) * 128],
                                pt_t[:, ib * JB:(ib + 1) * JB],
                                start=(ib == 0), stop=(ib == NIB - 1))
                    for cb4 in range(4):
                        cb = csup * 4 + cb4
                        ptp = pstp.tile([128, JB], F32, tag="ptp")
                        for jb in range(NJB):
                            wr_t = ior.tile([128, 128], F32, tag="wr_t")
                            nc.sync.dma_start(
                                wr_t[:],
                                wr_d[jb * 128:(jb + 1) * 128,
                                     cb * 128:(cb + 1) * 128])
                            nc.tensor.transpose(
                                ptp[:, jb * 128:(jb + 1) * 128], wr_t[:], ident[:])
                        wt_sb = ioo.tile([128, JB], F32, tag="wt_sb")
                        nc.scalar.activation(
                            wt_sb[:], ptp[:],
                            mybir.ActivationFunctionType.Identity,
                            bias=0.0, scale=i2_t[:, cb:cb + 1])
                        o_sb = ioo.tile([128, JB], F32, tag="o_sb")
                        nc.vector.tensor_mul(o_sb[:], pso[cb4][:], wt_sb[:])
                        nc.sync.dma_start(out_d[cb * 128:(cb + 1) * 128, :], o_sb[:])
    nc.finalize()
    return nc


_NC1 = None
_NC2 = None


def _get_ncs():
    global _NC1, _NC2
    if _NC1 is None:
        _NC1 = _build_neff1()
        _NC2 = _build_neff2()
    return _NC1, _NC2


def _ensure_trace_hook():
    """Best-effort NTFF profiling shim (test harness only; grading runs
    without tracing). The agent image's antenv lacks axon_hooks, but the
    axon boot package exposes the ctypes equivalent."""
    try:
        from antenv.axon_hooks import get_axon_ntff_profile_hook
        return get_axon_ntff_profile_hook() is not None
    except ImportError:
        pass
    try:
        import types
        if "/root/.axon_site" not in sys.path:
            sys.path.insert(0, "/root/.axon_site")
        from trn_agent_boot.trn_boot import _ntff_profile_via_ctypes
        hook = _ntff_profile_via_ctypes("/opt/axon/libaxon_pjrt.so")
        if hook is None:
            return False
        import antenv
        mod = types.ModuleType("antenv.axon_hooks")
        mod.get_axon_ntff_profile_hook = lambda: hook
        mod.set_axon_ntff_profile_hook = lambda h: None
        sys.modules["antenv.axon_hooks"] = mod
        antenv.axon_hooks = mod
        from concourse import bass_utils as _bu
        _bu.upload_artifacts = lambda tmpdir: ""
        return True
    except Exception:
        return False


def _run(nc, in_maps, cores, trace, tag):
    if trace:
        try:
            r = run_bass_kernel_spmd(nc, in_maps, cores, trace=True)
            LAST_EXEC[tag] = r.exec_time_ns
            LAST_RESULTS[tag] = r
            return r
        except Exception as e:
            print(f"trace run failed ({e!r}); retrying without trace")
    return run_bass_kernel_spmd(nc, in_maps, cores)


def kernel(node_features, adjacency_matrix, mask_father, neighbor_count,
           mask_hadamard, linear_w, linear_b, weight):
    nc1, nc2 = _get_ncs()
    trace = bool(int(os.environ.get("BASS_KERNEL_TRACE", "0"))) and _ensure_trace_hook()
    cores = list(range(M))
    bf = ml_dtypes.bfloat16

    nf = np.ascontiguousarray(np.asarray(node_features, dtype=np.float32))
    A = np.ascontiguousarray(np.asarray(adjacency_matrix, dtype=np.float32))
    Ao = np.ascontiguousarray(np.asarray(mask_father, dtype=np.float32)[:, 0, :])
    S = np.ascontiguousarray(np.asarray(mask_hadamard, dtype=np.float32)[:, 0, :])
    ncnt = np.asarray(neighbor_count, dtype=np.float32)
    lw = np.asarray(linear_w, dtype=np.float32)
    lb = np.asarray(linear_b, dtype=np.float32)
    W = np.ascontiguousarray(np.asarray(weight, dtype=np.float32))

    # ---- launch 1: wf rows ----
    lwT = np.ascontiguousarray(lw.T)                       # [F_RAW, IN_F]
    bias = np.ascontiguousarray(lb.reshape(IN_F // 128, 128).T)  # [128, 8]
    in1 = []
    for m in range(M):
        nfT = np.ascontiguousarray(nf[m * JB:(m + 1) * JB, :].T)  # [F_RAW, JB]
        in1.append({"lwT": lwT, "nfT": nfT, "bias": bias, "w": W})
    r1 = _run(nc1, in1, cores, trace, "neff1")
    wf = np.concatenate([r1.results[m]["wf_rows"] for m in range(M)], axis=0)

    # ---- launch 2: graph conv ----
    A_b = A.astype(bf)
    inv2 = (1.0 / np.square(ncnt.astype(np.float64)))[:, 0].astype(np.float32)
    inv2_t = np.ascontiguousarray(inv2.reshape(N // 128, 128).T)  # [128, 32]
    in2 = []
    for m in range(M):
        sl = slice(m * JB, (m + 1) * JB)
        in2.append({
            "a": A_b,
            "ao": np.ascontiguousarray(Ao[:, sl]).astype(bf),
            "s": np.ascontiguousarray(S[:, sl]).astype(bf),
            "wfd": wf,
            "wfs": np.ascontiguousarray(wf[sl, :]),
            "inv2": inv2_t,
        })
    r2 = _run(nc2, in2, cores, trace, "neff2")

    out = np.empty((C, N), dtype=np.float32)
    for m in range(M):
        out[:, m * JB:(m + 1) * JB] = r2.results[m]["outc"]
    return out



# revision 2
# speedup vs baseline: 1.3679x; 1.3679x over previous
"""Distributed Bass kernel for nn_Interaction_GraphConvolution.

Math (reference):
    x  = node_features @ linear_w.T + linear_b          [N, IN_F]
    wf = x @ weight                                     [N, C]
    G  = mask_father[:,0,:].T @ adjacency               [N, N]
    P  = G * mask_hadamard[:,0,:].T                     [N, N]
    out[c, j] = wf[j,c] * (P @ wf)[j,c] / neighbor_count[c]^2

Sharding: node dim j split across 8 cores, 512 each (J_m).
Two SPMD launches:
  NEFF-A: core m computes wf rows J_m AND G rows J_m (= gt). The two are
    independent, so the G GEMM (A in fp8, exact for 0/1 entries, resident
    in SBUF) streams back-to-back after the wf GEMMs keeping the PE warm.
  NEFF-B: core m computes PT[:, J_m] = gt.T * S[:, J_m] on-device, then
    PS rows J_m via stationary-PT / moving-wf matmuls (psum in [j, c]
    orientation), fused epilogue multiply with host-prescaled
    wf[J_m,:]*inv(ncnt^2).
Host between launches only reshapes/casts (gather wf, transpose gt,
slice S columns, fold inv2 into the elementwise factor); final output
is the per-core [j, c] block transposed into out[:, J_m].
Dtypes: A-side matmuls fp8e4 (0/1 exact); wf GEMM + PS GEMM bf16
(measured end-to-end max rel err ~3e-3 vs 2e-2 tolerance).
"""

import os
import sys

sys.path.insert(0, "/opt/trn_rl_repo")

import numpy as np
import ml_dtypes

from concourse import bass, bacc, mybir, tile
from concourse.bass_utils import run_bass_kernel_spmd

F32 = mybir.dt.float32
F32R = mybir.dt.float32r
BF16 = mybir.dt.bfloat16
F8 = mybir.dt.float8e4

N = 4096       # nodes (== out channels C)
F_RAW = 512    # raw feature dim
IN_F = 1024    # hidden dim
C = 4096       # out channels
M = 8          # cores
JB = N // M    # 512 nodes per core

NKB = N // 128   # 32 k-blocks
NIB = N // 128   # 32 i-blocks
NJB = JB // 128  # 4 j-blocks
NFB = IN_F // 128  # 8 f-blocks
NRB = F_RAW // 128  # 4 r-blocks

LAST_EXEC = {}
LAST_RESULTS = {}


def _build_neffA():
    """Per core m: wf rows J_m and G rows J_m.

    Inputs: lwT [F_RAW, IN_F] f32r, nfT [F_RAW, JB] f32r, bias [128, 8] f32,
    wb [IN_F, C] bf16, a8 [N, N] fp8 (adjacency), ac8 [N, JB] fp8
    (mask_father cols J_m).
    Outputs: wf_rows [JB, C] f32, gt [JB, N] bf16 (G[J_m, :]).
    """
    nc = bacc.Bacc()
    lwT_d = nc.dram_tensor("lwT", [F_RAW, IN_F], F32R, kind="ExternalInput")
    nfT_d = nc.dram_tensor("nfT", [F_RAW, JB], F32R, kind="ExternalInput")
    b_d = nc.dram_tensor("bias", [128, NFB], F32, kind="ExternalInput")
    wb_d = nc.dram_tensor("wb", [IN_F, C], BF16, kind="ExternalInput")
    a_d = nc.dram_tensor("a8", [N, N], F8, kind="ExternalInput")
    ac_d = nc.dram_tensor("ac8", [N, JB], F8, kind="ExternalInput")
    wf_d = nc.dram_tensor("wf_rows", [JB, C], F32, kind="ExternalOutput")
    gt_d = nc.dram_tensor("gt", [JB, N], BF16, kind="ExternalOutput")

    with tile.TileContext(nc) as tc:
        with tc.tile_pool(name="big", bufs=1) as bigp:
            # full adjacency, fp8, SBUF-resident (16MB): moving operand of G
            a_t = bigp.tile([128, NKB * N], F8)
            for kb in range(NKB):
                nc.gpsimd.dma_start(
                    a_t[:, kb * N:(kb + 1) * N],
                    a_d[kb * 128:(kb + 1) * 128, :])
            # mask_father cols J_m: stationary operand of G
            ac_t = bigp.tile([128, NKB * JB], F8)
            for kb in range(NKB):
                nc.sync.dma_start(
                    ac_t[:, kb * JB:(kb + 1) * JB],
                    ac_d[kb * 128:(kb + 1) * 128, :])
            # xT[f, j] in bf16, built by phase X
            xt_t = bigp.tile([128, NFB * JB], BF16)

            # ---- phase X: xT = lw @ nf[J_m].T + b ----
            with tc.tile_pool(name="xin", bufs=1) as xinp, \
                 tc.tile_pool(name="psx", bufs=2, space=bass.MemorySpace.PSUM) as psxp:
                lwT_t = xinp.tile([128, NRB * IN_F], F32R)
                for rb in range(NRB):
                    nc.sync.dma_start(
                        lwT_t[:, rb * IN_F:(rb + 1) * IN_F],
                        lwT_d[rb * 128:(rb + 1) * 128, :])
                nfT_t = xinp.tile([128, NRB * JB], F32R)
                for rb in range(NRB):
                    nc.sync.dma_start(
                        nfT_t[:, rb * JB:(rb + 1) * JB],
                        nfT_d[rb * 128:(rb + 1) * 128, :])
                b_t = xinp.tile([128, NFB], F32)
                nc.sync.dma_start(b_t[:], b_d[:])
                for fb in range(NFB):
                    psx = psxp.tile([128, JB], F32, tag="psx")
                    for rb in range(NRB):
                        nc.tensor.matmul(
                            psx[:],
                            lwT_t[:, rb * IN_F + fb * 128: rb * IN_F + (fb + 1) * 128],
                            nfT_t[:, rb * JB:(rb + 1) * JB],
                            start=(rb == 0), stop=(rb == NRB - 1))
                    nc.scalar.activation(
                        xt_t[:, fb * JB:(fb + 1) * JB], psx[:],
                        mybir.ActivationFunctionType.Identity,
                        bias=b_t[:, fb:fb + 1], scale=1.0)

            # ---- phase W: wf[J_m] = xT.T @ W (bf16) ----
            with tc.tile_pool(name="wst", bufs=2) as wstp, \
                 tc.tile_pool(name="psw", bufs=8, space=bass.MemorySpace.PSUM) as pswp, \
                 tc.tile_pool(name="wout", bufs=4) as woutp:
                for ch in range(C // 512):
                    wb_t = wstp.tile([128, NFB * 512], BF16, tag="wb")
                    for fb in range(NFB):
                        nc.sync.dma_start(
                            wb_t[:, fb * 512:(fb + 1) * 512],
                            wb_d[fb * 128:(fb + 1) * 128, ch * 512:(ch + 1) * 512])
                    pw = [pswp.tile([128, 512], F32, tag="pw", name=f"pw{_i}")
                          for _i in range(NJB)]
                    for fb in range(NFB):
                        for jb in range(NJB):
                            nc.tensor.matmul(
                                pw[jb][:],
                                xt_t[:, fb * JB + jb * 128: fb * JB + (jb + 1) * 128],
                                wb_t[:, fb * 512:(fb + 1) * 512],
                                start=(fb == 0), stop=(fb == NFB - 1))
                    for jb in range(NJB):
                        o_sb = woutp.tile([128, 512], F32, tag="o_sb")
                        nc.vector.tensor_copy(o_sb[:], pw[jb][:])
                        nc.sync.dma_start(
                            wf_d[jb * 128:(jb + 1) * 128, ch * 512:(ch + 1) * 512],
                            o_sb[:])

            # ---- phase G: gt = ac.T @ a (fp8, exact) ----
            with tc.tile_pool(name="psg", bufs=4, space=bass.MemorySpace.PSUM) as psgp, \
                 tc.tile_pool(name="gout", bufs=4) as goutp:
                for q in range(NJB):
                    for chunk in range(N // 512):
                        pg = psgp.tile([128, 512], F32, tag="pg")
                        for kb in range(NKB):
                            nc.tensor.matmul(
                                pg[:],
                                ac_t[:, kb * JB + q * 128: kb * JB + (q + 1) * 128],
                                a_t[:, kb * N + chunk * 512: kb * N + (chunk + 1) * 512],
                                start=(kb == 0), stop=(kb == NKB - 1))
                        g_sb = goutp.tile([128, 512], BF16, tag="g_sb")
                        nc.vector.tensor_copy(g_sb[:], pg[:])
                        nc.sync.dma_start(
                            gt_d[q * 128:(q + 1) * 128, chunk * 512:(chunk + 1) * 512],
                            g_sb[:])
    nc.finalize()
    return nc


def _build_neffB():
    """Per core m: PS rows J_m and fused epilogue.

    Inputs: wfd [N, C] bf16 (full wf), gtc [N, JB] bf16 (gt.T = G[J_m,:].T),
    sc [N, JB] bf16 (S cols J_m), ws2 [JB, C] f32 (wf[J_m,:] * inv2[c]).
    Output: ob [JB, C] f32 with ob[j, c] = out[c, J_m[j]].
    """
    nc = bacc.Bacc()
    wfd_d = nc.dram_tensor("wfd", [N, C], BF16, kind="ExternalInput")
    gtc_d = nc.dram_tensor("gtc", [N, JB], BF16, kind="ExternalInput")
    sc_d = nc.dram_tensor("sc", [N, JB], BF16, kind="ExternalInput")
    ws2_d = nc.dram_tensor("ws2", [JB, C], F32, kind="ExternalInput")
    ob_d = nc.dram_tensor("ob", [JB, C], F32, kind="ExternalOutput")

    NCQ = 4          # c-quarters
    CQ = C // NCQ    # 1024

    with tile.TileContext(nc) as tc:
        with tc.tile_pool(name="pt", bufs=1) as ptp:
            # PT[:, J_m] = gt.T * S[:, J_m], bf16, stationary for O phase
            pt_t = ptp.tile([128, NIB * JB], BF16)
            with tc.tile_pool(name="gs", bufs=4) as gsp:
                for ib in range(NIB):
                    g_t = gsp.tile([128, JB], BF16, tag="g_t")
                    nc.sync.dma_start(g_t[:], gtc_d[ib * 128:(ib + 1) * 128, :])
                    s_t = gsp.tile([128, JB], BF16, tag="s_t")
                    nc.sync.dma_start(s_t[:], sc_d[ib * 128:(ib + 1) * 128, :])
                    nc.vector.tensor_mul(
                        pt_t[:, ib * JB:(ib + 1) * JB], g_t[:], s_t[:])

            with tc.tile_pool(name="wq", bufs=2) as wqp, \
                 tc.tile_pool(name="psb", bufs=4, space=bass.MemorySpace.PSUM) as psbp, \
                 tc.tile_pool(name="w2p", bufs=3) as w2p, \
                 tc.tile_pool(name="eout", bufs=3) as eoutp:
                for cq in range(NCQ):
                    wq_t = wqp.tile([128, NIB * CQ], BF16, tag="wq")
                    for ib in range(NIB):
                        nc.scalar.dma_start(
                            wq_t[:, ib * CQ:(ib + 1) * CQ],
                            wfd_d[ib * 128:(ib + 1) * 128, cq * CQ:(cq + 1) * CQ])
                    for jb in range(NJB):
                        po = psbp.tile([128, CQ], F32, tag="po")
                        for ib in range(NIB):
                            st = pt_t[:, ib * JB + jb * 128: ib * JB + (jb + 1) * 128]
                            nc.tensor.matmul(
                                po[:, 0:512], st,
                                wq_t[:, ib * CQ: ib * CQ + 512],
                                start=(ib == 0), stop=(ib == NIB - 1))
                            nc.tensor.matmul(
                                po[:, 512:1024], st,
                                wq_t[:, ib * CQ + 512: (ib + 1) * CQ],
                                start=(ib == 0), stop=(ib == NIB - 1))
                        w2_t = w2p.tile([128, CQ], F32, tag="w2")
                        nc.sync.dma_start(
                            w2_t[:],
                            ws2_d[jb * 128:(jb + 1) * 128, cq * CQ:(cq + 1) * CQ])
                        o_sb = eoutp.tile([128, CQ], F32, tag="o_sb")
                        nc.vector.tensor_mul(o_sb[:], po[:], w2_t[:])
                        nc.sync.dma_start(
                            ob_d[jb * 128:(jb + 1) * 128, cq * CQ:(cq + 1) * CQ],
                            o_sb[:])
    nc.finalize()
    return nc


_NC1 = None
_NC2 = None


def _get_ncs():
    global _NC1, _NC2
    if _NC1 is None:
        _NC1 = _build_neffA()
        _NC2 = _build_neffB()
    return _NC1, _NC2


def _ensure_trace_hook():
    """Best-effort NTFF profiling shim (test harness only; grading runs
    without tracing). The agent image's antenv lacks axon_hooks, but the
    axon boot package exposes the ctypes equivalent."""
    try:
        from antenv.axon_hooks import get_axon_ntff_profile_hook
        return get_axon_ntff_profile_hook() is not None
    except ImportError:
        pass
    try:
        import types
        if "/root/.axon_site" not in sys.path:
            sys.path.insert(0, "/root/.axon_site")
        from trn_agent_boot.trn_boot import _ntff_profile_via_ctypes
        hook = _ntff_profile_via_ctypes("/opt/axon/libaxon_pjrt.so")
        if hook is None:
            return False
        import antenv
        mod = types.ModuleType("antenv.axon_hooks")
        mod.get_axon_ntff_profile_hook = lambda: hook
        mod.set_axon_ntff_profile_hook = lambda h: None
        sys.modules["antenv.axon_hooks"] = mod
        antenv.axon_hooks = mod
        from concourse import bass_utils as _bu
        _bu.upload_artifacts = lambda tmpdir: ""
        return True
    except Exception:
        return False


def _run(nc, in_maps, cores, trace, tag):
    if trace:
        try:
            r = run_bass_kernel_spmd(nc, in_maps, cores, trace=True)
            LAST_EXEC[tag] = r.exec_time_ns
            LAST_RESULTS[tag] = r
            return r
        except Exception as e:
            print(f"trace run failed ({e!r}); retrying without trace")
    return run_bass_kernel_spmd(nc, in_maps, cores)


def kernel(node_features, adjacency_matrix, mask_father, neighbor_count,
           mask_hadamard, linear_w, linear_b, weight):
    nc1, nc2 = _get_ncs()
    trace = bool(int(os.environ.get("BASS_KERNEL_TRACE", "0"))) and _ensure_trace_hook()
    cores = list(range(M))
    bf = ml_dtypes.bfloat16
    f8 = ml_dtypes.float8_e4m3fn

    nf = np.ascontiguousarray(np.asarray(node_features, dtype=np.float32))
    A = np.ascontiguousarray(np.asarray(adjacency_matrix, dtype=np.float32))
    Ao = np.ascontiguousarray(np.asarray(mask_father, dtype=np.float32)[:, 0, :])
    S = np.ascontiguousarray(np.asarray(mask_hadamard, dtype=np.float32)[:, 0, :])
    ncnt = np.asarray(neighbor_count, dtype=np.float32)
    lw = np.asarray(linear_w, dtype=np.float32)
    lb = np.asarray(linear_b, dtype=np.float32)
    W = np.ascontiguousarray(np.asarray(weight, dtype=np.float32))

    # ---- launch A: wf rows + G rows ----
    lwT = np.ascontiguousarray(lw.T)                       # [F_RAW, IN_F]
    bias = np.ascontiguousarray(lb.reshape(IN_F // 128, 128).T)  # [128, 8]
    wbB = W.astype(bf)
    a8 = A.astype(f8)
    in1 = []
    for m in range(M):
        sl = slice(m * JB, (m + 1) * JB)
        in1.append({
            "lwT": lwT,
            "nfT": np.ascontiguousarray(nf[sl, :].T),
            "bias": bias,
            "wb": wbB,
            "a8": a8,
            "ac8": np.ascontiguousarray(Ao[:, sl]).astype(f8),
        })
    r1 = _run(nc1, in1, cores, trace, "neffA")
    wf = np.concatenate([r1.results[m]["wf_rows"] for m in range(M)], axis=0)

    # ---- host reshaping (no compute beyond casts/elementwise staging) ----
    inv2 = (1.0 / np.square(ncnt.astype(np.float64)))[:, 0].astype(np.float32)
    wfd = wf.astype(bf)
    in2 = []
    for m in range(M):
        sl = slice(m * JB, (m + 1) * JB)
        in2.append({
            "wfd": wfd,
            "gtc": np.ascontiguousarray(r1.results[m]["gt"].T),   # [N, JB] bf16
            "sc": np.ascontiguousarray(S[:, sl]).astype(bf),
            "ws2": np.ascontiguousarray(wf[sl, :] * inv2[None, :]),
        })
    r2 = _run(nc2, in2, cores, trace, "neffB")

    out = np.empty((C, N), dtype=np.float32)
    for m in range(M):
        out[:, m * JB:(m + 1) * JB] = r2.results[m]["ob"].T
    return out


# revision 4
# speedup vs baseline: 1.7988x; 1.3150x over previous
"""Distributed Bass kernel for nn_Interaction_GraphConvolution.

Math (reference):
    x  = node_features @ linear_w.T + linear_b          [N, IN_F]
    wf = x @ weight                                     [N, C]
    G  = mask_father[:,0,:].T @ adjacency               [N, N]
    P  = G * mask_hadamard[:,0,:].T                     [N, N]
    out[c, j] = wf[j,c] * (P @ wf)[j,c] / neighbor_count[c]^2

Sharding: node dim j split across 8 cores, 512 each (J_m).
Two SPMD launches:
  NEFF-A: core m computes wf rows J_m AND G rows J_m (= gt). The two are
    independent, so the G GEMM (fp8 DoubleRow: adjacency 0/1 entries are
    exact in fp8, A resident in SBUF) streams back-to-back after the wf
    GEMMs keeping the PE warm.
  NEFF-B: core m computes PT[:, J_m] = gt.T * S[:, J_m] on-device, then
    PS rows J_m via stationary-PT / moving-wf matmuls (psum in [j, c]
    orientation), fused epilogue multiply with host-prescaled
    wf[J_m,:]*inv(ncnt^2).
Host between launches only reshapes/casts (gather wf, transpose gt,
slice S columns, fold inv2 into the elementwise factor); final output
is the per-core [j, c] block transposed into out[:, J_m].
Startup latency hiding: inputs land via few large strided DMA triggers
(critical-path tensors first), and a dummy-matmul warmup burst keeps the
PE HAM clock-gate at full rate while the first inputs stream in.
Dtypes: A-side matmuls fp8e4 (0/1 exact); wf GEMM + PS GEMM bf16
(measured end-to-end max rel err ~3e-3 vs 2e-2 tolerance).
"""

import os
import sys

sys.path.insert(0, "/opt/trn_rl_repo")

import numpy as np
import ml_dtypes

from concourse import bass, bacc, mybir, tile
from concourse.bass_utils import run_bass_kernel_spmd

F32 = mybir.dt.float32
F32R = mybir.dt.float32r
BF16 = mybir.dt.bfloat16
F8 = mybir.dt.float8e4

N = 4096       # nodes (== out channels C)
F_RAW = 512    # raw feature dim
IN_F = 1024    # hidden dim
C = 4096       # out channels
M = 8          # cores
JB = N // M    # 512 nodes per core

NKB = N // 128   # 32 k-blocks
NIB = N // 128   # 32 i-blocks
NJB = JB // 128  # 4 j-blocks
NFB = IN_F // 128  # 8 f-blocks
NRB = F_RAW // 128  # 4 r-blocks

DOUBLE_ROW = True   # fp8 DoubleRow for the G GEMM (2 k-blocks per matmul)

LAST_EXEC = {}
LAST_RESULTS = {}


def _warmup(nc, tc, n_mm=40):
    """Dummy matmul burst: keeps the PE busy (HAM stays at full clock)
    while the first real inputs stream in from HBM."""
    with tc.tile_pool(name="warm", bufs=1) as wp, \
         tc.tile_pool(name="pswarm", bufs=1, space=bass.MemorySpace.PSUM) as pwp:
        wtile = wp.tile([128, 512], BF16)
        nc.gpsimd.memset(wtile[:], 1.0)
        pwarm = pwp.tile([128, 512], F32, tag="pwarm")
        for _ in range(n_mm):
            nc.tensor.matmul(pwarm[:], wtile[:, 0:128], wtile[:],
                             start=True, stop=True)


def _build_neffA():
    """Per core m: wf rows J_m and G rows J_m.

    Inputs: lwT [F_RAW, IN_F] f32r, nfT [F_RAW, JB] f32r, bias [128, 8] f32,
    wb [IN_F, C] bf16, a8 [N, N] fp8 (adjacency), ac8 [N, JB] fp8
    (mask_father cols J_m).
    Outputs: wf_rows [JB, C] f32, gt [JB, N] bf16 (G[J_m, :]).
    """
    nc = bacc.Bacc()
    lwT_d = nc.dram_tensor("lwT", [F_RAW, IN_F], F32R, kind="ExternalInput")
    nfT_d = nc.dram_tensor("nfT", [F_RAW, JB], F32R, kind="ExternalInput")
    b_d = nc.dram_tensor("bias", [128, NFB], F32, kind="ExternalInput")
    wb_d = nc.dram_tensor("wb", [IN_F, C], BF16, kind="ExternalInput")
    a_d = nc.dram_tensor("a8", [N, N], F8, kind="ExternalInput")
    ac_d = nc.dram_tensor("ac8", [N, JB], F8, kind="ExternalInput")
    wf_d = nc.dram_tensor("wf_rows", [JB, C], F32, kind="ExternalOutput")
    gt_d = nc.dram_tensor("gt", [JB, N], BF16, kind="ExternalOutput")

    with tile.TileContext(nc) as tc:
        _warmup(nc, tc)
        with tc.tile_pool(name="big", bufs=1) as bigp:
            # full adjacency, fp8, SBUF-resident (16MB): moving operand of G.
            # Single trigger on the scalar HWDGE queue so it never head-of-line
            # blocks the critical-path inputs on the sync queue.
            a_t = bigp.tile([128, NKB, N], F8)
            nc.scalar.dma_start(
                a_t[:], a_d[:].rearrange("(kb p) n -> p kb n", p=128))
            xt_t = bigp.tile([128, NFB, JB], BF16)

            # ---- phase X: xT = lw @ nf[J_m].T + b ----
            with tc.tile_pool(name="xin", bufs=1) as xinp, \
                 tc.tile_pool(name="psx", bufs=2, space=bass.MemorySpace.PSUM) as psxp:
                lwT_t = xinp.tile([128, NRB, IN_F], F32R)
                nc.sync.dma_start(
                    lwT_t[:], lwT_d[:].rearrange("(rb p) f -> p rb f", p=128))
                nfT_t = xinp.tile([128, NRB, JB], F32R)
                nc.sync.dma_start(
                    nfT_t[:], nfT_d[:].rearrange("(rb p) j -> p rb j", p=128))
                b_t = xinp.tile([128, NFB], F32)
                nc.sync.dma_start(b_t[:], b_d[:])
                for fb in range(NFB):
                    psx = psxp.tile([128, JB], F32, tag="psx")
                    for rb in range(NRB):
                        nc.tensor.matmul(
                            psx[:],
                            lwT_t[:, rb, fb * 128:(fb + 1) * 128],
                            nfT_t[:, rb, :],
                            start=(rb == 0), stop=(rb == NRB - 1))
                    nc.scalar.activation(
                        xt_t[:, fb, :], psx[:],
                        mybir.ActivationFunctionType.Identity,
                        bias=b_t[:, fb:fb + 1], scale=1.0)

            # ---- phase W: wf[J_m] = xT.T @ W (bf16) ----
            with tc.tile_pool(name="wst", bufs=2) as wstp, \
                 tc.tile_pool(name="psw", bufs=8, space=bass.MemorySpace.PSUM) as pswp, \
                 tc.tile_pool(name="wout", bufs=4) as woutp:
                for ch in range(C // 512):
                    wb_t = wstp.tile([128, NFB, 512], BF16, tag="wb")
                    nc.sync.dma_start(
                        wb_t[:],
                        wb_d[:, ch * 512:(ch + 1) * 512].rearrange(
                            "(fb p) c -> p fb c", p=128))
                    pw = [pswp.tile([128, 512], F32, tag="pw", name=f"pw{_i}")
                          for _i in range(NJB)]
                    for fb in range(NFB):
                        for jb in range(NJB):
                            nc.tensor.matmul(
                                pw[jb][:],
                                xt_t[:, fb, jb * 128:(jb + 1) * 128],
                                wb_t[:, fb, :],
                                start=(fb == 0), stop=(fb == NFB - 1))
                    for jb in range(NJB):
                        o_sb = woutp.tile([128, 512], F32, tag="o_sb")
                        nc.vector.tensor_copy(o_sb[:], pw[jb][:])
                        nc.sync.dma_start(
                            wf_d[jb * 128:(jb + 1) * 128, ch * 512:(ch + 1) * 512],
                            o_sb[:])

            # ---- phase G: gt = ac.T @ a (fp8, exact) ----
            with tc.tile_pool(name="gin", bufs=1) as ginp, \
                 tc.tile_pool(name="psg", bufs=4, space=bass.MemorySpace.PSUM) as psgp, \
                 tc.tile_pool(name="gout", bufs=4) as goutp:
                ac_t = ginp.tile([128, NKB, JB], F8)
                nc.sync.dma_start(
                    ac_t[:], ac_d[:].rearrange("(kb p) j -> p kb j", p=128))
                for q in range(NJB):
                    for chunk in range(N // 512):
                        pg = psgp.tile([128, 512], F32, tag="pg")
                        if DOUBLE_ROW:
                            for kt in range(NKB // 2):
                                nc.tensor.matmul(
                                    pg[:],
                                    ac_t[:, 2 * kt:2 * kt + 2, q * 128:(q + 1) * 128],
                                    a_t[:, 2 * kt:2 * kt + 2, chunk * 512:(chunk + 1) * 512],
                                    start=(kt == 0), stop=(kt == NKB // 2 - 1),
                                    perf_mode=mybir.MatmulPerfMode.DoubleRow)
                        else:
                            for kb in range(NKB):
                                nc.tensor.matmul(
                                    pg[:],
                                    ac_t[:, kb, q * 128:(q + 1) * 128],
                                    a_t[:, kb, chunk * 512:(chunk + 1) * 512],
                                    start=(kb == 0), stop=(kb == NKB - 1))
                        g_sb = goutp.tile([128, 512], BF16, tag="g_sb")
                        nc.vector.tensor_copy(g_sb[:], pg[:])
                        nc.sync.dma_start(
                            gt_d[q * 128:(q + 1) * 128, chunk * 512:(chunk + 1) * 512],
                            g_sb[:])
    nc.finalize()
    return nc


def _build_neffB():
    """Per core m: PS rows J_m and fused epilogue.

    Inputs: wfd [N, C] bf16 (full wf), gtc [N, JB] bf16 (gt.T = G[J_m,:].T),
    sc [N, JB] bf16 (S cols J_m), ws2 [JB, C] f32 (wf[J_m,:] * inv2[c]).
    Output: ob [JB, C] f32 with ob[j, c] = out[c, J_m[j]].
    """
    nc = bacc.Bacc()
    wfd_d = nc.dram_tensor("wfd", [N, C], BF16, kind="ExternalInput")
    gtc_d = nc.dram_tensor("gtc", [N, JB], BF16, kind="ExternalInput")
    sc_d = nc.dram_tensor("sc", [N, JB], BF16, kind="ExternalInput")
    ws2_d = nc.dram_tensor("ws2", [JB, C], F32, kind="ExternalInput")
    ob_d = nc.dram_tensor("ob", [JB, C], F32, kind="ExternalOutput")

    NCQ = 4          # c-quarters
    CQ = C // NCQ    # 1024
    GCH = 4          # gc/sc DMA chunk: 4 i-blocks per trigger
    WCH = 8          # wfd DMA chunk: 8 i-blocks per trigger

    with tile.TileContext(nc) as tc:
        _warmup(nc, tc)
        with tc.tile_pool(name="pt", bufs=1) as ptp:
            # PT[:, J_m] = gt.T * S[:, J_m], bf16, stationary for O phase
            pt_t = ptp.tile([128, NIB, JB], BF16)
            with tc.tile_pool(name="gs", bufs=1) as gsp:
                g_t = gsp.tile([128, NIB, JB], BF16)
                s_t = gsp.tile([128, NIB, JB], BF16)
                for i0 in range(0, NIB, GCH):
                    nc.sync.dma_start(
                        g_t[:, i0:i0 + GCH, :],
                        gtc_d[i0 * 128:(i0 + GCH) * 128, :].rearrange(
                            "(ib p) j -> p ib j", p=128))
                    nc.sync.dma_start(
                        s_t[:, i0:i0 + GCH, :],
                        sc_d[i0 * 128:(i0 + GCH) * 128, :].rearrange(
                            "(ib p) j -> p ib j", p=128))
                for ib in range(NIB):
                    nc.vector.tensor_mul(
                        pt_t[:, ib, :], g_t[:, ib, :], s_t[:, ib, :])

            with tc.tile_pool(name="wq", bufs=2) as wqp, \
                 tc.tile_pool(name="psb", bufs=4, space=bass.MemorySpace.PSUM) as psbp, \
                 tc.tile_pool(name="w2p", bufs=3) as w2p, \
                 tc.tile_pool(name="eout", bufs=3) as eoutp:
                for cq in range(NCQ):
                    wq_t = wqp.tile([128, NIB, CQ], BF16, tag="wq")
                    for i0 in range(0, NIB, WCH):
                        nc.scalar.dma_start(
                            wq_t[:, i0:i0 + WCH, :],
                            wfd_d[i0 * 128:(i0 + WCH) * 128,
                                  cq * CQ:(cq + 1) * CQ].rearrange(
                                "(ib p) c -> p ib c", p=128))
                    for jb in range(NJB):
                        po = psbp.tile([128, CQ], F32, tag="po")
                        for ib in range(NIB):
                            st = pt_t[:, ib, jb * 128:(jb + 1) * 128]
                            nc.tensor.matmul(
                                po[:, 0:512], st,
                                wq_t[:, ib, 0:512],
                                start=(ib == 0), stop=(ib == NIB - 1))
                            nc.tensor.matmul(
                                po[:, 512:1024], st,
                                wq_t[:, ib, 512:1024],
                                start=(ib == 0), stop=(ib == NIB - 1))
                        w2_t = w2p.tile([128, CQ], F32, tag="w2")
                        nc.sync.dma_start(
                            w2_t[:],
                            ws2_d[jb * 128:(jb + 1) * 128, cq * CQ:(cq + 1) * CQ])
                        o_sb = eoutp.tile([128, CQ], F32, tag="o_sb")
                        nc.vector.tensor_mul(o_sb[:], po[:], w2_t[:])
                        nc.sync.dma_start(
                            ob_d[jb * 128:(jb + 1) * 128, cq * CQ:(cq + 1) * CQ],
                            o_sb[:])
    nc.finalize()
    return nc


_NC1 = None
_NC2 = None


def _get_ncs():
    global _NC1, _NC2
    if _NC1 is None:
        _NC1 = _build_neffA()
        _NC2 = _build_neffB()
    return _NC1, _NC2


def _ensure_trace_hook():
    """Best-effort NTFF profiling shim (test harness only; grading runs
    without tracing). The agent image's antenv lacks axon_hooks, but the
    axon boot package exposes the ctypes equivalent."""
    try:
        from antenv.axon_hooks import get_axon_ntff_profile_hook
        return get_axon_ntff_profile_hook() is not None
    except ImportError:
        pass
    try:
        import types
        if "/root/.axon_site" not in sys.path:
            sys.path.insert(0, "/root/.axon_site")
        from trn_agent_boot.trn_boot import _ntff_profile_via_ctypes
        hook = _ntff_profile_via_ctypes("/opt/axon/libaxon_pjrt.so")
        if hook is None:
            return False
        import antenv
        mod = types.ModuleType("antenv.axon_hooks")
        mod.get_axon_ntff_profile_hook = lambda: hook
        mod.set_axon_ntff_profile_hook = lambda h: None
        sys.modules["antenv.axon_hooks"] = mod
        antenv.axon_hooks = mod
        from concourse import bass_utils as _bu
        _bu.upload_artifacts = lambda tmpdir: ""
        return True
    except Exception:
        return False


def _run(nc, in_maps, cores, trace, tag):
    if trace:
        try:
            r = run_bass_kernel_spmd(nc, in_maps, cores, trace=True)
            LAST_EXEC[tag] = r.exec_time_ns
            LAST_RESULTS[tag] = r
            return r
        except Exception as e:
            print(f"trace run failed ({e!r}); retrying without trace")
    return run_bass_kernel_spmd(nc, in_maps, cores)


def kernel(node_features, adjacency_matrix, mask_father, neighbor_count,
           mask_hadamard, linear_w, linear_b, weight):
    nc1, nc2 = _get_ncs()
    trace = bool(int(os.environ.get("BASS_KERNEL_TRACE", "0"))) and _ensure_trace_hook()
    cores = list(range(M))
    bf = ml_dtypes.bfloat16
    f8 = ml_dtypes.float8_e4m3fn

    nf = np.ascontiguousarray(np.asarray(node_features, dtype=np.float32))
    A = np.ascontiguousarray(np.asarray(adjacency_matrix, dtype=np.float32))
    Ao = np.ascontiguousarray(np.asarray(mask_father, dtype=np.float32)[:, 0, :])
    S = np.ascontiguousarray(np.asarray(mask_hadamard, dtype=np.float32)[:, 0, :])
    ncnt = np.asarray(neighbor_count, dtype=np.float32)
    lw = np.asarray(linear_w, dtype=np.float32)
    lb = np.asarray(linear_b, dtype=np.float32)
    W = np.ascontiguousarray(np.asarray(weight, dtype=np.float32))

    # ---- launch A: wf rows + G rows ----
    lwT = np.ascontiguousarray(lw.T)                       # [F_RAW, IN_F]
    bias = np.ascontiguousarray(lb.reshape(IN_F // 128, 128).T)  # [128, 8]
    wbB = W.astype(bf)
    a8 = A.astype(f8)
    in1 = []
    for m in range(M):
        sl = slice(m * JB, (m + 1) * JB)
        in1.append({
            "lwT": lwT,
            "nfT": np.ascontiguousarray(nf[sl, :].T),
            "bias": bias,
            "wb": wbB,
            "a8": a8,
            "ac8": np.ascontiguousarray(Ao[:, sl]).astype(f8),
        })
    r1 = _run(nc1, in1, cores, trace, "neffA")
    wf = np.concatenate([r1.results[m]["wf_rows"] for m in range(M)], axis=0)

    # ---- host reshaping (no compute beyond casts/elementwise staging) ----
    inv2 = (1.0 / np.square(ncnt.astype(np.float64)))[:, 0].astype(np.float32)
    wfd = wf.astype(bf)
    in2 = []
    for m in range(M):
        sl = slice(m * JB, (m + 1) * JB)
        in2.append({
            "wfd": wfd,
            "gtc": np.ascontiguousarray(r1.results[m]["gt"].T),   # [N, JB] bf16
            "sc": np.ascontiguousarray(S[:, sl]).astype(bf),
            "ws2": np.ascontiguousarray(wf[sl, :] * inv2[None, :]),
        })
    r2 = _run(nc2, in2, cores, trace, "neffB")

    out = np.empty((C, N), dtype=np.float32)
    for m in range(M):
        out[:, m * JB:(m + 1) * JB] = r2.results[m]["ob"].T
    return out


# revision 5
# speedup vs baseline: 1.8667x; 1.0377x over previous
"""Distributed Bass kernel for nn_Interaction_GraphConvolution.

Math (reference):
    x  = node_features @ linear_w.T + linear_b          [N, IN_F]
    wf = x @ weight                                     [N, C]
    G  = mask_father[:,0,:].T @ adjacency               [N, N]
    P  = G * mask_hadamard[:,0,:].T                     [N, N]
    out[c, j] = wf[j,c] * (P @ wf)[j,c] / neighbor_count[c]^2

Sharding: node dim j split across 8 cores, 512 each (J_m).
Two SPMD launches:
  NEFF-A: core m computes wf rows J_m (Linear in f32r, projection GEMM in
    bf16), written out in bf16.
  NEFF-B: core m computes PT[:, J_m] = G[J_m,:].T * S[:, J_m] on-device
    (fp8 DoubleRow GEMM: adjacency 0/1 entries are exact in fp8; matmul
    oriented stationary=A-column-tiles / moving=mask_father-columns so
    the psum lands directly in [i, j] stationary orientation for the O
    phase - no transpose anywhere), then PS rows J_m via stationary-PT /
    moving-wf bf16 matmuls (psum in [j, c] orientation), with a fused
    epilogue multiply by the host-prescaled wf[J_m,:]*inv(ncnt^2).
Host between launches only reshapes/casts (gather wf, slice S columns,
fold inv2 into the elementwise factor); final output is the per-core
[j, c] block transposed into out[:, J_m].
Startup latency hiding: inputs land via few large strided DMA triggers
(critical-path tensors first, bulk tensors on a second queue), and a
dummy-matmul warmup burst keeps the PE HAM clock-gate at full rate
while the first inputs stream in.
Measured end-to-end max rel err ~4e-3 vs 2e-2 tolerance.
"""

import os
import sys

sys.path.insert(0, "/opt/trn_rl_repo")

import numpy as np
import ml_dtypes

from concourse import bass, bacc, mybir, tile
from concourse.bass_utils import run_bass_kernel_spmd

F32 = mybir.dt.float32
F32R = mybir.dt.float32r
BF16 = mybir.dt.bfloat16
F8 = mybir.dt.float8e4

N = 4096       # nodes (== out channels C)
F_RAW = 512    # raw feature dim
IN_F = 1024    # hidden dim
C = 4096       # out channels
M = 8          # cores
JB = N // M    # 512 nodes per core

NKB = N // 128   # 32 k-blocks
NIB = N // 128   # 32 i-blocks
NJB = JB // 128  # 4 j-blocks
NFB = IN_F // 128  # 8 f-blocks
NRB = F_RAW // 128  # 4 r-blocks

LAST_EXEC = {}
LAST_RESULTS = {}


def _warmup(nc, tc, n_mm=40):
    """Dummy matmul burst: keeps the PE busy (HAM stays at full clock)
    while the first real inputs stream in from HBM."""
    with tc.tile_pool(name="warm", bufs=1) as wp, \
         tc.tile_pool(name="pswarm", bufs=1, space=bass.MemorySpace.PSUM) as pwp:
        wtile = wp.tile([128, 512], BF16)
        nc.gpsimd.memset(wtile[:], 1.0)
        pwarm = pwp.tile([128, 512], F32, tag="pwarm")
        for _ in range(n_mm):
            nc.tensor.matmul(pwarm[:], wtile[:, 0:128], wtile[:],
                             start=True, stop=True)


def _build_neffA():
    """Per core m: wf rows J_m.

    Inputs: lwT [F_RAW, IN_F] f32r, nfT [F_RAW, JB] f32r, bias [128, 8] f32,
    wb [IN_F, C] bf16. Output: wf_rows [JB, C] bf16.
    """
    nc = bacc.Bacc()
    lwT_d = nc.dram_tensor("lwT", [F_RAW, IN_F], F32R, kind="ExternalInput")
    nfT_d = nc.dram_tensor("nfT", [F_RAW, JB], F32R, kind="ExternalInput")
    b_d = nc.dram_tensor("bias", [128, NFB], F32, kind="ExternalInput")
    wb_d = nc.dram_tensor("wb", [IN_F, C], BF16, kind="ExternalInput")
    wf_d = nc.dram_tensor("wf_rows", [JB, C], BF16, kind="ExternalOutput")

    with tile.TileContext(nc) as tc:
        _warmup(nc, tc)
        with tc.tile_pool(name="big", bufs=1) as bigp:
            xt_t = bigp.tile([128, NFB, JB], BF16)

            # ---- phase X: xT = lw @ nf[J_m].T + b ----
            with tc.tile_pool(name="xin", bufs=1) as xinp, \
                 tc.tile_pool(name="psx", bufs=2, space=bass.MemorySpace.PSUM) as psxp:
                lwT_t = xinp.tile([128, NRB, IN_F], F32R)
                nc.sync.dma_start(
                    lwT_t[:], lwT_d[:].rearrange("(rb p) f -> p rb f", p=128))
                nfT_t = xinp.tile([128, NRB, JB], F32R)
                nc.sync.dma_start(
                    nfT_t[:], nfT_d[:].rearrange("(rb p) j -> p rb j", p=128))
                b_t = xinp.tile([128, NFB], F32)
                nc.sync.dma_start(b_t[:], b_d[:])
                for fb in range(NFB):
                    psx = psxp.tile([128, JB], F32, tag="psx")
                    for rb in range(NRB):
                        nc.tensor.matmul(
                            psx[:],
                            lwT_t[:, rb, fb * 128:(fb + 1) * 128],
                            nfT_t[:, rb, :],
                            start=(rb == 0), stop=(rb == NRB - 1))
                    nc.scalar.activation(
                        xt_t[:, fb, :], psx[:],
                        mybir.ActivationFunctionType.Identity,
                        bias=b_t[:, fb:fb + 1], scale=1.0)

            # ---- phase W: wf[J_m] = xT.T @ W (bf16) ----
            with tc.tile_pool(name="wst", bufs=2) as wstp, \
                 tc.tile_pool(name="psw", bufs=8, space=bass.MemorySpace.PSUM) as pswp, \
                 tc.tile_pool(name="wout", bufs=4) as woutp:
                for ch in range(C // 512):
                    wb_t = wstp.tile([128, NFB, 512], BF16, tag="wb")
                    nc.sync.dma_start(
                        wb_t[:],
                        wb_d[:, ch * 512:(ch + 1) * 512].rearrange(
                            "(fb p) c -> p fb c", p=128))
                    pw = [pswp.tile([128, 512], F32, tag="pw", name=f"pw{_i}")
                          for _i in range(NJB)]
                    for fb in range(NFB):
                        for jb in range(NJB):
                            nc.tensor.matmul(
                                pw[jb][:],
                                xt_t[:, fb, jb * 128:(jb + 1) * 128],
                                wb_t[:, fb, :],
                                start=(fb == 0), stop=(fb == NFB - 1))
                    for jb in range(NJB):
                        o_sb = woutp.tile([128, 512], BF16, tag="o_sb")
                        nc.vector.tensor_copy(o_sb[:], pw[jb][:])
                        nc.sync.dma_start(
                            wf_d[jb * 128:(jb + 1) * 128, ch * 512:(ch + 1) * 512],
                            o_sb[:])
    nc.finalize()
    return nc


def _build_neffB():
    """Per core m: G rows J_m -> PT, PS rows J_m, fused epilogue.

    Inputs: a8 [N, N] fp8 (adjacency), ac8 [N, JB] fp8 (mask_father cols),
    sc [N, JB] bf16 (S cols J_m), wfd [N, C] bf16 (full wf),
    ws2 [JB, C] f32 (wf[J_m,:] * inv2[c]).
    Output: ob [JB, C] f32 with ob[j, c] = out[c, J_m[j]].
    """
    nc = bacc.Bacc()
    a_d = nc.dram_tensor("a8", [N, N], F8, kind="ExternalInput")
    ac_d = nc.dram_tensor("ac8", [N, JB], F8, kind="ExternalInput")
    sc_d = nc.dram_tensor("sc", [N, JB], BF16, kind="ExternalInput")
    wfd_d = nc.dram_tensor("wfd", [N, C], BF16, kind="ExternalInput")
    ws2_d = nc.dram_tensor("ws2", [JB, C], F32, kind="ExternalInput")
    ob_d = nc.dram_tensor("ob", [JB, C], F32, kind="ExternalOutput")

    NIQ = 4          # i-quarters for the A stream (G stationary)
    IQ = NIB // NIQ  # 8 i-blocks per quarter
    NCE = 8          # c-eighths for the wf stream (O moving)
    CE = C // NCE    # 512
    SCH = 4          # sc DMA chunk: 4 i-blocks per trigger

    with tile.TileContext(nc) as tc:
        _warmup(nc, tc)
        with tc.tile_pool(name="pt", bufs=1) as ptp, \
             tc.tile_pool(name="wq", bufs=2) as wqp:
            pt_t = ptp.tile([128, NIB, JB], BF16)
            # wfd streams on the scalar HWDGE queue while G runs on data
            # from the sync queue; the wq pool is opened before the G pools
            # so the transfers are not serialized behind G's SBUF reuse.
            wq_tiles = []
            for ce in range(NCE):
                wq_t = wqp.tile([128, NIB, CE], BF16, tag="wq")
                if ce < 2:
                    nc.scalar.dma_start(
                        wq_t[:],
                        wfd_d[:, ce * CE:(ce + 1) * CE].rearrange(
                            "(ib p) c -> p ib c", p=128))
                wq_tiles.append(wq_t)

            # ---- phase G: pt[i, j] = (A^T @ mf_cols)[i, j] * S[i, J_m[j]] ----
            with tc.tile_pool(name="acm", bufs=1) as acmp, \
                 tc.tile_pool(name="aq", bufs=2) as aqp, \
                 tc.tile_pool(name="scp", bufs=2) as scp, \
                 tc.tile_pool(name="psg", bufs=4, space=bass.MemorySpace.PSUM) as psgp:
                ac_t = acmp.tile([128, NKB, JB], F8)
                nc.sync.dma_start(
                    ac_t[:], ac_d[:].rearrange("(kb p) j -> p kb j", p=128))
                for iq in range(NIQ):
                    aq_t = aqp.tile([128, NKB, IQ * 128], F8, tag="aq")
                    nc.sync.dma_start(
                        aq_t[:],
                        a_d[:, iq * IQ * 128:(iq + 1) * IQ * 128].rearrange(
                            "(kb p) i -> p kb i", p=128))
                    for ib8 in range(IQ):
                        ib = iq * IQ + ib8
                        if ib % SCH == 0:
                            sc_t = scp.tile([128, SCH, JB], BF16, tag="sc")
                            nc.sync.dma_start(
                                sc_t[:],
                                sc_d[ib * 128:(ib + SCH) * 128, :].rearrange(
                                    "(b p) j -> p b j", p=128))
                        pg = psgp.tile([128, JB], F32, tag="pg")
                        for kt in range(NKB // 2):
                            nc.tensor.matmul(
                                pg[:],
                                aq_t[:, 2 * kt:2 * kt + 2, ib8 * 128:(ib8 + 1) * 128],
                                ac_t[:, 2 * kt:2 * kt + 2, :],
                                start=(kt == 0), stop=(kt == NKB // 2 - 1),
                                perf_mode=mybir.MatmulPerfMode.DoubleRow)
                        nc.vector.tensor_mul(
                            pt_t[:, ib, :], pg[:], sc_t[:, ib % SCH, :])

            # ---- phase O: ob[j, c] = (pt.T @ wf)[j, c] * ws2[j, c] ----
            with tc.tile_pool(name="psb", bufs=6, space=bass.MemorySpace.PSUM) as psbp, \
                 tc.tile_pool(name="w2p", bufs=3) as w2p, \
                 tc.tile_pool(name="eout", bufs=3) as eoutp:
                for ce in range(NCE):
                    wq_t = wq_tiles[ce]
                    if ce >= 2:
                        nc.scalar.dma_start(
                            wq_t[:],
                            wfd_d[:, ce * CE:(ce + 1) * CE].rearrange(
                                "(ib p) c -> p ib c", p=128))
                    for jb in range(NJB):
                        po = psbp.tile([128, CE], F32, tag="po")
                        for ib in range(NIB):
                            nc.tensor.matmul(
                                po[:],
                                pt_t[:, ib, jb * 128:(jb + 1) * 128],
                                wq_t[:, ib, :],
                                start=(ib == 0), stop=(ib == NIB - 1))
                        w2_t = w2p.tile([128, CE], F32, tag="w2")
                        nc.sync.dma_start(
                            w2_t[:],
                            ws2_d[jb * 128:(jb + 1) * 128, ce * CE:(ce + 1) * CE])
                        o_sb = eoutp.tile([128, CE], F32, tag="o_sb")
                        nc.vector.tensor_mul(o_sb[:], po[:], w2_t[:])
                        nc.sync.dma_start(
                            ob_d[jb * 128:(jb + 1) * 128, ce * CE:(ce + 1) * CE],
                            o_sb[:])
    nc.finalize()
    return nc


_NC1 = None
_NC2 = None


def _get_ncs():
    global _NC1, _NC2
    if _NC1 is None:
        _NC1 = _build_neffA()
        _NC2 = _build_neffB()
    return _NC1, _NC2


def _ensure_trace_hook():
    """Best-effort NTFF profiling shim (test harness only; grading runs
    without tracing). The agent image's antenv lacks axon_hooks, but the
    axon boot package exposes the ctypes equivalent."""
    try:
        from antenv.axon_hooks import get_axon_ntff_profile_hook
        return get_axon_ntff_profile_hook() is not None
    except ImportError:
        pass
    try:
        import types
        if "/root/.axon_site" not in sys.path:
            sys.path.insert(0, "/root/.axon_site")
        from trn_agent_boot.trn_boot import _ntff_profile_via_ctypes
        hook = _ntff_profile_via_ctypes("/opt/axon/libaxon_pjrt.so")
        if hook is None:
            return False
        import antenv
        mod = types.ModuleType("antenv.axon_hooks")
        mod.get_axon_ntff_profile_hook = lambda: hook
        mod.set_axon_ntff_profile_hook = lambda h: None
        sys.modules["antenv.axon_hooks"] = mod
        antenv.axon_hooks = mod
        from concourse import bass_utils as _bu
        _bu.upload_artifacts = lambda tmpdir: ""
        return True
    except Exception:
        return False


def _run(nc, in_maps, cores, trace, tag):
    if trace:
        try:
            r = run_bass_kernel_spmd(nc, in_maps, cores, trace=True)
            LAST_EXEC[tag] = r.exec_time_ns
            LAST_RESULTS[tag] = r
            return r
        except Exception as e:
            print(f"trace run failed ({e!r}); retrying without trace")
    return run_bass_kernel_spmd(nc, in_maps, cores)


def kernel(node_features, adjacency_matrix, mask_father, neighbor_count,
           mask_hadamard, linear_w, linear_b, weight):
    nc1, nc2 = _get_ncs()
    trace = bool(int(os.environ.get("BASS_KERNEL_TRACE", "0"))) and _ensure_trace_hook()
    cores = list(range(M))
    bf = ml_dtypes.bfloat16
    f8 = ml_dtypes.float8_e4m3fn

    nf = np.ascontiguousarray(np.asarray(node_features, dtype=np.float32))
    A = np.ascontiguousarray(np.asarray(adjacency_matrix, dtype=np.float32))
    Ao = np.ascontiguousarray(np.asarray(mask_father, dtype=np.float32)[:, 0, :])
    S = np.ascontiguousarray(np.asarray(mask_hadamard, dtype=np.float32)[:, 0, :])
    ncnt = np.asarray(neighbor_count, dtype=np.float32)
    lw = np.asarray(linear_w, dtype=np.float32)
    lb = np.asarray(linear_b, dtype=np.float32)
    W = np.ascontiguousarray(np.asarray(weight, dtype=np.float32))

    # ---- launch A: wf rows ----
    lwT = np.ascontiguousarray(lw.T)                       # [F_RAW, IN_F]
    bias = np.ascontiguousarray(lb.reshape(IN_F // 128, 128).T)  # [128, 8]
    wbB = W.astype(bf)
    in1 = []
    for m in range(M):
        sl = slice(m * JB, (m + 1) * JB)
        in1.append({
            "lwT": lwT,
            "nfT": np.ascontiguousarray(nf[sl, :].T),
            "bias": bias,
            "wb": wbB,
        })
    r1 = _run(nc1, in1, cores, trace, "neffA")
    wfd = np.concatenate([r1.results[m]["wf_rows"] for m in range(M)], axis=0)
    wf32 = wfd.astype(np.float32)

    # ---- host reshaping (no compute beyond casts/elementwise staging) ----
    inv2 = (1.0 / np.square(ncnt.astype(np.float64)))[:, 0].astype(np.float32)
    a8 = A.astype(f8)
    in2 = []
    for m in range(M):
        sl = slice(m * JB, (m + 1) * JB)
        in2.append({
            "a8": a8,
            "ac8": np.ascontiguousarray(Ao[:, sl]).astype(f8),
            "sc": np.ascontiguousarray(S[:, sl]).astype(bf),
            "wfd": wfd,
            "ws2": np.ascontiguousarray(wf32[sl, :] * inv2[None, :]),
        })
    r2 = _run(nc2, in2, cores, trace, "neffB")

    out = np.empty((C, N), dtype=np.float32)
    for m in range(M):
        out[:, m * JB:(m + 1) * JB] = r2.results[m]["ob"].T
    return out


# revision 10
# speedup vs baseline: 1.8878x; 1.0113x over previous
"""Distributed Bass kernel for nn_Interaction_GraphConvolution.

Math (reference):
    x  = node_features @ linear_w.T + linear_b          [N, IN_F]
    wf = x @ weight                                     [N, C]
    G  = mask_father[:,0,:].T @ adjacency               [N, N]
    P  = G * mask_hadamard[:,0,:].T                     [N, N]
    out[c, j] = wf[j,c] * (P @ wf)[j,c] / neighbor_count[c]^2

Sharding: node dim j split across 8 cores, 512 each (J_m).
Two SPMD launches:
  NEFF-A: core m computes wf rows J_m (Linear in f32r, projection GEMM in
    bf16), written out in bf16.
  NEFF-B: core m computes PT[:, J_m] = G[J_m,:].T * S[:, J_m] on-device
    (fp8 DoubleRow GEMM: adjacency 0/1 entries are exact in fp8; matmul
    oriented stationary=A-column-tiles / moving=mask_father-columns so
    the psum lands directly in [i, j] stationary orientation for the O
    phase - no transpose anywhere), then PS rows J_m via stationary-PT /
    moving-wf bf16 matmuls (psum in [j, c] orientation), with a fused
    epilogue multiply by the host-prescaled wf[J_m,:]*inv(ncnt^2).
Host between launches only reshapes/casts (gather wf, slice S columns,
fold inv2 into the elementwise factor); final output is the per-core
[j, c] block transposed into out[:, J_m].
Startup latency hiding: inputs land via few large strided DMA triggers
(critical-path tensors first, bulk tensors on a second queue), and a
dummy-matmul warmup burst keeps the PE HAM clock-gate at full rate
while the first inputs stream in.
Measured end-to-end max rel err ~4e-3 vs 2e-2 tolerance.
"""

import os
import sys

sys.path.insert(0, "/opt/trn_rl_repo")

import numpy as np
import ml_dtypes

from concourse import bass, bacc, mybir, tile
from concourse.bass_utils import run_bass_kernel_spmd

F32 = mybir.dt.float32
F32R = mybir.dt.float32r
BF16 = mybir.dt.bfloat16
F8 = mybir.dt.float8e4

N = 4096       # nodes (== out channels C)
F_RAW = 512    # raw feature dim
IN_F = 1024    # hidden dim
C = 4096       # out channels
M = 8          # cores
JB = N // M    # 512 nodes per core

NKB = N // 128   # 32 k-blocks
NIB = N // 128   # 32 i-blocks
NJB = JB // 128  # 4 j-blocks
NFB = IN_F // 128  # 8 f-blocks
NRB = F_RAW // 128  # 4 r-blocks

LAST_EXEC = {}
LAST_RESULTS = {}


def _warmup(nc, tc, n_mm=40):
    """Dummy matmul burst: keeps the PE busy (HAM stays at full clock)
    while the first real inputs stream in from HBM."""
    with tc.tile_pool(name="warm", bufs=1) as wp, \
         tc.tile_pool(name="pswarm", bufs=1, space=bass.MemorySpace.PSUM) as pwp:
        wtile = wp.tile([128, 512], BF16)
        nc.gpsimd.memset(wtile[:], 1.0)
        pwarm = pwp.tile([128, 512], F32, tag="pwarm")
        for _ in range(n_mm):
            nc.tensor.matmul(pwarm[:], wtile[:, 0:128], wtile[:],
                             start=True, stop=True)


def _build_neffA():
    """Per core m: wf rows J_m.

    Inputs: lwT [F_RAW, IN_F] f32r, nfT [F_RAW, JB] f32r, bias [128, 8] f32,
    wb [IN_F, C] bf16. Output: wf_rows [JB, C] bf16.
    """
    nc = bacc.Bacc()
    lwT_d = nc.dram_tensor("lwT", [F_RAW, IN_F], F32R, kind="ExternalInput")
    nfT_d = nc.dram_tensor("nfT", [F_RAW, JB], F32R, kind="ExternalInput")
    b_d = nc.dram_tensor("bias", [128, NFB], F32, kind="ExternalInput")
    wb_d = nc.dram_tensor("wb", [IN_F, C], BF16, kind="ExternalInput")
    wf_d = nc.dram_tensor("wf_rows", [JB, C], BF16, kind="ExternalOutput")

    with tile.TileContext(nc) as tc:
        _warmup(nc, tc)
        with tc.tile_pool(name="big", bufs=1) as bigp:
            xt_t = bigp.tile([128, NFB, JB], BF16)

            # ---- phase X: xT = lw @ nf[J_m].T + b ----
            with tc.tile_pool(name="xin", bufs=1) as xinp, \
                 tc.tile_pool(name="psx", bufs=2, space=bass.MemorySpace.PSUM) as psxp:
                lwT_t = xinp.tile([128, NRB, IN_F], F32R)
                nc.sync.dma_start(
                    lwT_t[:], lwT_d[:].rearrange("(rb p) f -> p rb f", p=128))
                nfT_t = xinp.tile([128, NRB, JB], F32R)
                nc.sync.dma_start(
                    nfT_t[:], nfT_d[:].rearrange("(rb p) j -> p rb j", p=128))
                b_t = xinp.tile([128, NFB], F32)
                nc.sync.dma_start(b_t[:], b_d[:])
                for fb in range(NFB):
                    psx = psxp.tile([128, JB], F32, tag="psx")
                    for rb in range(NRB):
                        nc.tensor.matmul(
                            psx[:],
                            lwT_t[:, rb, fb * 128:(fb + 1) * 128],
                            nfT_t[:, rb, :],
                            start=(rb == 0), stop=(rb == NRB - 1))
                    nc.scalar.activation(
                        xt_t[:, fb, :], psx[:],
                        mybir.ActivationFunctionType.Identity,
                        bias=b_t[:, fb:fb + 1], scale=1.0)

            # ---- phase W: wf[J_m] = xT.T @ W (bf16) ----
            # psw gets 6 banks so it never aliases psx's 2: the first W
            # matmuls (fb=0) can then overlap phase X's tail.
            with tc.tile_pool(name="wst", bufs=2) as wstp, \
                 tc.tile_pool(name="psw", bufs=6, space=bass.MemorySpace.PSUM) as pswp, \
                 tc.tile_pool(name="wout", bufs=4) as woutp:
                for ch in range(C // 512):
                    wb_t = wstp.tile([128, NFB, 512], BF16, tag="wb")
                    nc.sync.dma_start(
                        wb_t[:],
                        wb_d[:, ch * 512:(ch + 1) * 512].rearrange(
                            "(fb p) c -> p fb c", p=128))
                    pw = [pswp.tile([128, 512], F32, tag="pw", name=f"pw{_i}")
                          for _i in range(NJB)]
                    for fb in range(NFB):
                        for jb in range(NJB):
                            nc.tensor.matmul(
                                pw[jb][:],
                                xt_t[:, fb, jb * 128:(jb + 1) * 128],
                                wb_t[:, fb, :],
                                start=(fb == 0), stop=(fb == NFB - 1))
                    for jb in range(NJB):
                        o_sb = woutp.tile([128, 512], BF16, tag="o_sb")
                        nc.vector.tensor_copy(o_sb[:], pw[jb][:])
                        nc.sync.dma_start(
                            wf_d[jb * 128:(jb + 1) * 128, ch * 512:(ch + 1) * 512],
                            o_sb[:])
    nc.finalize()
    return nc


def _build_neffB():
    """Per core m: G rows J_m -> PT, PS rows J_m, fused epilogue.

    Inputs: a8 [N, N] fp8 (adjacency), ac8 [N, JB] fp8 (mask_father cols),
    sc [N, JB] bf16 (S cols J_m), wfd [N, C] bf16 (full wf),
    ws2 [JB, C] f32 (wf[J_m,:] * inv2[c]).
    Output: ob [JB, C] f32 with ob[j, c] = out[c, J_m[j]].
    """
    nc = bacc.Bacc()
    a_d = nc.dram_tensor("a8", [N, N], F8, kind="ExternalInput")
    ac_d = nc.dram_tensor("ac8", [N, JB], F8, kind="ExternalInput")
    sc_d = nc.dram_tensor("sc", [N, JB], BF16, kind="ExternalInput")
    wfd_d = nc.dram_tensor("wfd", [N, C], BF16, kind="ExternalInput")
    ws2_d = nc.dram_tensor("ws2", [JB, C], F32, kind="ExternalInput")
    ob_d = nc.dram_tensor("ob", [JB, C], F32, kind="ExternalOutput")

    NIQ = 4          # i-quarters for the A stream (G stationary)
    IQ = NIB // NIQ  # 8 i-blocks per quarter
    NCE = 8          # c-eighths for the wf stream (O moving)
    CE = C // NCE    # 512
    SCH = 4          # sc DMA chunk: 4 i-blocks per trigger

    with tile.TileContext(nc) as tc:
        _warmup(nc, tc)
        with tc.tile_pool(name="pt", bufs=1) as ptp, \
             tc.tile_pool(name="wq", bufs=2) as wqp:
            pt_t = ptp.tile([128, NIB, JB], BF16)
            # wfd streams on the scalar HWDGE queue while G runs on data
            # from the sync queue; the wq pool is opened before the G pools
            # so the transfers are not serialized behind G's SBUF reuse.
            # The first two prefetches are issued late in phase G so they
            # don't compete with G's own input stream at launch.
            wq_tiles = [wqp.tile([128, NIB, CE], BF16, tag="wq", name=f"wq{_i}")
                        for _i in range(NCE)]

            def _wq_fetch(ce):
                nc.scalar.dma_start(
                    wq_tiles[ce][:],
                    wfd_d[:, ce * CE:(ce + 1) * CE].rearrange(
                        "(ib p) c -> p ib c", p=128))

            # ---- phase G: pt[i, j] = (A^T @ mf_cols)[i, j] * S[i, J_m[j]] ----
            with tc.tile_pool(name="acm", bufs=1) as acmp, \
                 tc.tile_pool(name="aq", bufs=2) as aqp, \
                 tc.tile_pool(name="scp", bufs=2) as scp, \
                 tc.tile_pool(name="psg", bufs=4, space=bass.MemorySpace.PSUM) as psgp:
                HK = NKB // 2 * 128   # half the k rows per trigger
                ac_t = acmp.tile([128, NKB, JB], F8)
                for h in range(2):
                    nc.sync.dma_start(
                        ac_t[:, h * (NKB // 2):(h + 1) * (NKB // 2), :],
                        ac_d[h * HK:(h + 1) * HK, :].rearrange(
                            "(kb p) j -> p kb j", p=128))
                for iq in range(NIQ):
                    aq_t = aqp.tile([128, NKB, IQ * 128], F8, tag="aq")
                    for h in range(2):
                        nc.sync.dma_start(
                            aq_t[:, h * (NKB // 2):(h + 1) * (NKB // 2), :],
                            a_d[h * HK:(h + 1) * HK,
                                iq * IQ * 128:(iq + 1) * IQ * 128].rearrange(
                                "(kb p) i -> p kb i", p=128))
                    if iq == NIQ - 1:
                        _wq_fetch(0)
                        _wq_fetch(1)
                    for ib8 in range(IQ):
                        ib = iq * IQ + ib8
                        if ib % SCH == 0:
                            sc_t = scp.tile([128, SCH, JB], BF16, tag="sc")
                            nc.sync.dma_start(
                                sc_t[:],
                                sc_d[ib * 128:(ib + SCH) * 128, :].rearrange(
                                    "(b p) j -> p b j", p=128))
                        pg = psgp.tile([128, JB], F32, tag="pg")
                        for kt in range(NKB // 2):
                            nc.tensor.matmul(
                                pg[:],
                                aq_t[:, 2 * kt:2 * kt + 2, ib8 * 128:(ib8 + 1) * 128],
                                ac_t[:, 2 * kt:2 * kt + 2, :],
                                start=(kt == 0), stop=(kt == NKB // 2 - 1),
                                perf_mode=mybir.MatmulPerfMode.DoubleRow)
                        nc.vector.tensor_mul(
                            pt_t[:, ib, :], pg[:], sc_t[:, ib % SCH, :])

            # ---- phase O: ob[j, c] = (pt.T @ wf)[j, c] * ws2[j, c] ----
            with tc.tile_pool(name="psb", bufs=6, space=bass.MemorySpace.PSUM) as psbp, \
                 tc.tile_pool(name="w2p", bufs=3) as w2p, \
                 tc.tile_pool(name="eout", bufs=3) as eoutp:
                for ce in range(NCE):
                    wq_t = wq_tiles[ce]
                    if ce >= 2:
                        _wq_fetch(ce)
                    for jb in range(NJB):
                        po = psbp.tile([128, CE], F32, tag="po")
                        for ib in range(NIB):
                            nc.tensor.matmul(
                                po[:],
                                pt_t[:, ib, jb * 128:(jb + 1) * 128],
                                wq_t[:, ib, :],
                                start=(ib == 0), stop=(ib == NIB - 1))
                        w2_t = w2p.tile([128, CE], F32, tag="w2")
                        nc.sync.dma_start(
                            w2_t[:],
                            ws2_d[jb * 128:(jb + 1) * 128, ce * CE:(ce + 1) * CE])
                        o_sb = eoutp.tile([128, CE], F32, tag="o_sb")
                        nc.vector.tensor_mul(o_sb[:], po[:], w2_t[:])
                        nc.sync.dma_start(
                            ob_d[jb * 128:(jb + 1) * 128, ce * CE:(ce + 1) * CE],
                            o_sb[:])
    nc.finalize()
    return nc


_NC1 = None
_NC2 = None


def _get_ncs():
    global _NC1, _NC2
    if _NC1 is None:
        _NC1 = _build_neffA()
        _NC2 = _build_neffB()
    return _NC1, _NC2


def _ensure_trace_hook():
    """Best-effort NTFF profiling shim (test harness only; grading runs
    without tracing). The agent image's antenv lacks axon_hooks, but the
    axon boot package exposes the ctypes equivalent."""
    try:
        from antenv.axon_hooks import get_axon_ntff_profile_hook
        return get_axon_ntff_profile_hook() is not None
    except ImportError:
        pass
    try:
        import types
        if "/root/.axon_site" not in sys.path:
            sys.path.insert(0, "/root/.axon_site")
        from trn_agent_boot.trn_boot import _ntff_profile_via_ctypes
        hook = _ntff_profile_via_ctypes("/opt/axon/libaxon_pjrt.so")
        if hook is None:
            return False
        import antenv
        mod = types.ModuleType("antenv.axon_hooks")
        mod.get_axon_ntff_profile_hook = lambda: hook
        mod.set_axon_ntff_profile_hook = lambda h: None
        sys.modules["antenv.axon_hooks"] = mod
        antenv.axon_hooks = mod
        from concourse import bass_utils as _bu
        _bu.upload_artifacts = lambda tmpdir: ""
        return True
    except Exception:
        return False


def _run(nc, in_maps, cores, trace, tag):
    if trace:
        try:
            r = run_bass_kernel_spmd(nc, in_maps, cores, trace=True)
            LAST_EXEC[tag] = r.exec_time_ns
            LAST_RESULTS[tag] = r
            return r
        except Exception as e:
            print(f"trace run failed ({e!r}); retrying without trace")
    return run_bass_kernel_spmd(nc, in_maps, cores)


def kernel(node_features, adjacency_matrix, mask_father, neighbor_count,
           mask_hadamard, linear_w, linear_b, weight):
    nc1, nc2 = _get_ncs()
    trace = bool(int(os.environ.get("BASS_KERNEL_TRACE", "0"))) and _ensure_trace_hook()
    cores = list(range(M))
    bf = ml_dtypes.bfloat16
    f8 = ml_dtypes.float8_e4m3fn

    nf = np.ascontiguousarray(np.asarray(node_features, dtype=np.float32))
    A = np.ascontiguousarray(np.asarray(adjacency_matrix, dtype=np.float32))
    Ao = np.ascontiguousarray(np.asarray(mask_father, dtype=np.float32)[:, 0, :])
    S = np.ascontiguousarray(np.asarray(mask_hadamard, dtype=np.float32)[:, 0, :])
    ncnt = np.asarray(neighbor_count, dtype=np.float32)
    lw = np.asarray(linear_w, dtype=np.float32)
    lb = np.asarray(linear_b, dtype=np.float32)
    W = np.ascontiguousarray(np.asarray(weight, dtype=np.float32))

    # ---- launch A: wf rows ----
    lwT = np.ascontiguousarray(lw.T)                       # [F_RAW, IN_F]
    bias = np.ascontiguousarray(lb.reshape(IN_F // 128, 128).T)  # [128, 8]
    wbB = W.astype(bf)
    in1 = []
    for m in range(M):
        sl = slice(m * JB, (m + 1) * JB)
        in1.append({
            "lwT": lwT,
            "nfT": np.ascontiguousarray(nf[sl, :].T),
            "bias": bias,
            "wb": wbB,
        })
    r1 = _run(nc1, in1, cores, trace, "neffA")
    wfd = np.concatenate([r1.results[m]["wf_rows"] for m in range(M)], axis=0)
    wf32 = wfd.astype(np.float32)

    # ---- host reshaping (no compute beyond casts/elementwise staging) ----
    inv2 = (1.0 / np.square(ncnt.astype(np.float64)))[:, 0].astype(np.float32)
    a8 = A.astype(f8)
    in2 = []
    for m in range(M):
        sl = slice(m * JB, (m + 1) * JB)
        in2.append({
            "a8": a8,
            "ac8": np.ascontiguousarray(Ao[:, sl]).astype(f8),
            "sc": np.ascontiguousarray(S[:, sl]).astype(bf),
            "wfd": wfd,
            "ws2": np.ascontiguousarray(wf32[sl, :] * inv2[None, :]),
        })
    r2 = _run(nc2, in2, cores, trace, "neffB")

    out = np.empty((C, N), dtype=np.float32)
    for m in range(M):
        out[:, m * JB:(m + 1) * JB] = r2.results[m]["ob"].T
    return out


# revision 17
# speedup vs baseline: 1.9319x; 1.0234x over previous
"""Distributed Bass kernel for nn_Interaction_GraphConvolution.

Math (reference):
    x  = node_features @ linear_w.T + linear_b          [N, IN_F]
    wf = x @ weight                                     [N, C]
    G  = mask_father[:,0,:].T @ adjacency               [N, N]
    P  = G * mask_hadamard[:,0,:].T                     [N, N]
    out[c, j] = wf[j,c] * (P @ wf)[j,c] / neighbor_count[c]^2

Sharding: node dim j split across 8 cores, 512 each (J_m).
Two SPMD launches:
  NEFF-A: core m computes wf rows J_m (Linear in f32r, projection GEMM in
    bf16), written out in bf16.
  NEFF-B: core m computes PT[:, J_m] = G[J_m,:].T * S[:, J_m] on-device
    (fp8 DoubleRow GEMM: adjacency 0/1 entries are exact in fp8; matmul
    oriented stationary=A-column-tiles / moving=mask_father-columns so
    the psum lands directly in [i, j] stationary orientation for the O
    phase - no transpose anywhere), then PS rows J_m via stationary-PT /
    moving-wf bf16 matmuls (psum in [j, c] orientation), with a fused
    epilogue multiply by the host-prescaled wf[J_m,:]*inv(ncnt^2).
Host between launches only reshapes/casts (gather wf, slice S columns,
fold inv2 into the elementwise factor); final output is the per-core
[j, c] block transposed into out[:, J_m].
Startup latency hiding: inputs land via few large strided DMA triggers
(critical-path tensors first, bulk tensors on a second queue), and a
dummy-matmul warmup burst keeps the PE HAM clock-gate at full rate
while the first inputs stream in.
Measured end-to-end max rel err ~4e-3 vs 2e-2 tolerance.
"""

import os
import sys

sys.path.insert(0, "/opt/trn_rl_repo")

import numpy as np
import ml_dtypes

from concourse import bass, bacc, mybir, tile
from concourse.bass_utils import run_bass_kernel_spmd

F32 = mybir.dt.float32
F32R = mybir.dt.float32r
BF16 = mybir.dt.bfloat16
F8 = mybir.dt.float8e4

N = 4096       # nodes (== out channels C)
F_RAW = 512    # raw feature dim
IN_F = 1024    # hidden dim
C = 4096       # out channels
M = 8          # cores
JB = N // M    # 512 nodes per core

NKB = N // 128   # 32 k-blocks
NIB = N // 128   # 32 i-blocks
NJB = JB // 128  # 4 j-blocks
NFB = IN_F // 128  # 8 f-blocks
NRB = F_RAW // 128  # 4 r-blocks

LAST_EXEC = {}
LAST_RESULTS = {}


def _warmup(nc, tc, n_mm=40):
    """Dummy matmul burst: keeps the PE busy (HAM stays at full clock)
    while the first real inputs stream in from HBM."""
    with tc.tile_pool(name="warm", bufs=1) as wp, \
         tc.tile_pool(name="pswarm", bufs=1, space=bass.MemorySpace.PSUM) as pwp:
        wtile = wp.tile([128, 512], BF16)
        nc.gpsimd.memset(wtile[:], 1.0)
        pwarm = pwp.tile([128, 512], F32, tag="pwarm")
        for _ in range(n_mm):
            nc.tensor.matmul(pwarm[:], wtile[:, 0:128], wtile[:],
                             start=True, stop=True)


def _build_neffA():
    """Per core m: wf rows J_m.

    Inputs: lwT [F_RAW, IN_F] f32r, nfT [F_RAW, JB] f32r, bias [128, 8] f32,
    wb [IN_F, C] bf16. Output: wf_rows [JB, C] bf16.
    """
    nc = bacc.Bacc()
    lwT_d = nc.dram_tensor("lwT", [F_RAW, IN_F], F32R, kind="ExternalInput")
    nfT_d = nc.dram_tensor("nfT", [F_RAW, JB], F32R, kind="ExternalInput")
    b_d = nc.dram_tensor("bias", [128, NFB], F32, kind="ExternalInput")
    # wbq[ch*128+p, fb*512+j] = W[fb*128+p, ch*512+j]: host-swizzled so each
    # 512-column chunk loads as one fully-contiguous DMA
    wb_d = nc.dram_tensor("wbq", [IN_F, C], BF16, kind="ExternalInput")
    wf_d = nc.dram_tensor("wf_rows", [JB, C], BF16, kind="ExternalOutput")

    with tile.TileContext(nc) as tc:
        _warmup(nc, tc)
        with tc.tile_pool(name="big", bufs=1) as bigp:
            xt_t = bigp.tile([128, NFB, JB], BF16)

            # ---- phase X: xT = lw @ nf[J_m].T + b ----
            with tc.tile_pool(name="xin", bufs=1) as xinp, \
                 tc.tile_pool(name="psx", bufs=2, space=bass.MemorySpace.PSUM) as psxp:
                lwT_t = xinp.tile([128, NRB, IN_F], F32R)
                nc.sync.dma_start(
                    lwT_t[:], lwT_d[:].rearrange("(rb p) f -> p rb f", p=128))
                nfT_t = xinp.tile([128, NRB, JB], F32R)
                nc.sync.dma_start(
                    nfT_t[:], nfT_d[:].rearrange("(rb p) j -> p rb j", p=128))
                b_t = xinp.tile([128, NFB], F32)
                nc.sync.dma_start(b_t[:], b_d[:])
                for fb in range(NFB):
                    psx = psxp.tile([128, JB], F32, tag="psx")
                    for rb in range(NRB):
                        nc.tensor.matmul(
                            psx[:],
                            lwT_t[:, rb, fb * 128:(fb + 1) * 128],
                            nfT_t[:, rb, :],
                            start=(rb == 0), stop=(rb == NRB - 1))
                    nc.scalar.activation(
                        xt_t[:, fb, :], psx[:],
                        mybir.ActivationFunctionType.Identity,
                        bias=b_t[:, fb:fb + 1], scale=1.0)

            # ---- phase W: wf[J_m] = xT.T @ W (bf16) ----
            # psw gets 6 banks so it never aliases psx's 2: the first W
            # matmuls (fb=0) can then overlap phase X's tail.
            with tc.tile_pool(name="wst", bufs=2) as wstp, \
                 tc.tile_pool(name="psw", bufs=6, space=bass.MemorySpace.PSUM) as pswp, \
                 tc.tile_pool(name="wout", bufs=4) as woutp:
                for ch in range(C // 512):
                    wb_t = wstp.tile([128, NFB, 512], BF16, tag="wb")
                    nc.sync.dma_start(
                        wb_t[:],
                        wb_d[ch * 128:(ch + 1) * 128, :].rearrange(
                            "p (fb c) -> p fb c", fb=NFB))
                    pw = [pswp.tile([128, 512], F32, tag="pw", name=f"pw{_i}")
                          for _i in range(NJB)]
                    for fb in range(NFB):
                        for jb in range(NJB):
                            nc.tensor.matmul(
                                pw[jb][:],
                                xt_t[:, fb, jb * 128:(jb + 1) * 128],
                                wb_t[:, fb, :],
                                start=(fb == 0), stop=(fb == NFB - 1))
                    for jb in range(NJB):
                        o_sb = woutp.tile([128, 512], BF16, tag="o_sb")
                        nc.vector.tensor_copy(o_sb[:], pw[jb][:])
                        nc.sync.dma_start(
                            wf_d[jb * 128:(jb + 1) * 128, ch * 512:(ch + 1) * 512],
                            o_sb[:])
    nc.finalize()
    return nc


def _build_neffB():
    """Per core m: G rows J_m -> PT, PS rows J_m, fused epilogue.

    Inputs: a8 [N, N] fp8 (adjacency), ac8 [N, JB] fp8 (mask_father cols),
    sc [N, JB] bf16 (S cols J_m), wfd [N, C] bf16 (full wf),
    ws2 [JB, C] f32 (wf[J_m,:] * inv2[c]).
    Output: ob [JB, C] f32 with ob[j, c] = out[c, J_m[j]].
    """
    nc = bacc.Bacc()
    # All bulk inputs are host-swizzled so every DMA reads fully-contiguous
    # lines (strided sub-2KB lines halve effective HBM bandwidth):
    #   a8q [iq*128+p, kb*1024+i'] = A[kb*128+p, iq*1024+i']
    #   acq [p, kb*512+j]          = mf[kb*128+p, J_m[j]]
    #   scq [p, ib*512+j]          = S[ib*128+p, J_m[j]]
    #   wfq [ce*128+p, ib*512+c']  = wf[ib*128+p, ce*512+c']
    a_d = nc.dram_tensor("a8q", [4 * 128, NKB * 1024], F8, kind="ExternalInput")
    ac_d = nc.dram_tensor("acq", [128, NKB * JB], F8, kind="ExternalInput")
    sc_d = nc.dram_tensor("scq", [128, NIB * JB], BF16, kind="ExternalInput")
    wfd_d = nc.dram_tensor("wfq", [8 * 128, NIB * 512], BF16, kind="ExternalInput")
    ws2_d = nc.dram_tensor("ws2", [JB, C], F32, kind="ExternalInput")
    ob_d = nc.dram_tensor("ob", [JB, C], F32, kind="ExternalOutput")

    NIQ = 4          # i-quarters for the A stream (G stationary)
    IQ = NIB // NIQ  # 8 i-blocks per quarter
    NCE = 8          # c-eighths for the wf stream (O moving)
    CE = C // NCE    # 512
    SCH = 4          # sc DMA chunk: 4 i-blocks per trigger

    with tile.TileContext(nc) as tc:
        _warmup(nc, tc)
        with tc.tile_pool(name="pt", bufs=1) as ptp, \
             tc.tile_pool(name="wq", bufs=2) as wqp:
            pt_t = ptp.tile([128, NIB, JB], BF16)
            # wfd streams on the scalar HWDGE queue while G runs on data
            # from the sync queue; the wq pool is opened before the G pools
            # so the transfers are not serialized behind G's SBUF reuse.
            # The first two prefetches are issued late in phase G so they
            # don't compete with G's own input stream at launch.
            wq_tiles = [wqp.tile([128, NIB, CE], BF16, tag="wq", name=f"wq{_i}")
                        for _i in range(NCE)]

            def _wq_fetch(ce):
                nc.scalar.dma_start(
                    wq_tiles[ce][:],
                    wfd_d[ce * 128:(ce + 1) * 128, :].rearrange(
                        "p (ib c) -> p ib c", ib=NIB))

            # ---- phase G: pt[i, j] = (A^T @ mf_cols)[i, j] * S[i, J_m[j]] ----
            with tc.tile_pool(name="acm", bufs=1) as acmp, \
                 tc.tile_pool(name="aq", bufs=2) as aqp, \
                 tc.tile_pool(name="scp", bufs=2) as scp, \
                 tc.tile_pool(name="psg", bufs=4, space=bass.MemorySpace.PSUM) as psgp:
                HKC = NKB // 2 * 1024   # half the columns of one a8q row-block
                ac_t = acmp.tile([128, NKB, JB], F8)
                for h in range(2):
                    nc.sync.dma_start(
                        ac_t[:, h * (NKB // 2):(h + 1) * (NKB // 2), :],
                        ac_d[:, h * (NKB // 2) * JB:(h + 1) * (NKB // 2) * JB]
                        .rearrange("p (kb j) -> p kb j", kb=NKB // 2))
                for iq in range(NIQ):
                    aq_t = aqp.tile([128, NKB, IQ * 128], F8, tag="aq")
                    for h in range(2):
                        nc.sync.dma_start(
                            aq_t[:, h * (NKB // 2):(h + 1) * (NKB // 2), :],
                            a_d[iq * 128:(iq + 1) * 128, h * HKC:(h + 1) * HKC]
                            .rearrange("p (kb i) -> p kb i", kb=NKB // 2))
                    if iq == NIQ - 1:
                        _wq_fetch(0)
                        _wq_fetch(1)
                    for ib8 in range(IQ):
                        ib = iq * IQ + ib8
                        if ib % SCH == 0:
                            sc_t = scp.tile([128, SCH, JB], BF16, tag="sc")
                            nc.sync.dma_start(
                                sc_t[:],
                                sc_d[:, ib * JB:(ib + SCH) * JB].rearrange(
                                    "p (b j) -> p b j", b=SCH))
                        pg = psgp.tile([128, JB], F32, tag="pg")
                        for kt in range(NKB // 2):
                            nc.tensor.matmul(
                                pg[:],
                                aq_t[:, 2 * kt:2 * kt + 2, ib8 * 128:(ib8 + 1) * 128],
                                ac_t[:, 2 * kt:2 * kt + 2, :],
                                start=(kt == 0), stop=(kt == NKB // 2 - 1),
                                perf_mode=mybir.MatmulPerfMode.DoubleRow)
                        nc.vector.tensor_mul(
                            pt_t[:, ib, :], pg[:], sc_t[:, ib % SCH, :])

            # ---- phase O: ob[j, c] = (pt.T @ wf)[j, c] * ws2[j, c] ----
            with tc.tile_pool(name="psb", bufs=6, space=bass.MemorySpace.PSUM) as psbp, \
                 tc.tile_pool(name="w2p", bufs=3) as w2p, \
                 tc.tile_pool(name="eout", bufs=3) as eoutp:
                for ce in range(NCE):
                    wq_t = wq_tiles[ce]
                    if ce >= 2:
                        _wq_fetch(ce)
                    for jb in range(NJB):
                        po = psbp.tile([128, CE], F32, tag="po")
                        for ib in range(NIB):
                            nc.tensor.matmul(
                                po[:],
                                pt_t[:, ib, jb * 128:(jb + 1) * 128],
                                wq_t[:, ib, :],
                                start=(ib == 0), stop=(ib == NIB - 1))
                        w2_t = w2p.tile([128, CE], F32, tag="w2")
                        nc.sync.dma_start(
                            w2_t[:],
                            ws2_d[jb * 128:(jb + 1) * 128, ce * CE:(ce + 1) * CE])
                        o_sb = eoutp.tile([128, CE], F32, tag="o_sb")
                        nc.vector.tensor_mul(o_sb[:], po[:], w2_t[:])
                        nc.sync.dma_start(
                            ob_d[jb * 128:(jb + 1) * 128, ce * CE:(ce + 1) * CE],
                            o_sb[:])
    nc.finalize()
    return nc


_NC1 = None
_NC2 = None


def _get_ncs():
    global _NC1, _NC2
    if _NC1 is None:
        _NC1 = _build_neffA()
        _NC2 = _build_neffB()
    return _NC1, _NC2


def _ensure_trace_hook():
    """Best-effort NTFF profiling shim (test harness only; grading runs
    without tracing). The agent image's antenv lacks axon_hooks, but the
    axon boot package exposes the ctypes equivalent."""
    try:
        from antenv.axon_hooks import get_axon_ntff_profile_hook
        return get_axon_ntff_profile_hook() is not None
    except ImportError:
        pass
    try:
        import types
        if "/root/.axon_site" not in sys.path:
            sys.path.insert(0, "/root/.axon_site")
        from trn_agent_boot.trn_boot import _ntff_profile_via_ctypes
        hook = _ntff_profile_via_ctypes("/opt/axon/libaxon_pjrt.so")
        if hook is None:
            return False
        import antenv
        mod = types.ModuleType("antenv.axon_hooks")
        mod.get_axon_ntff_profile_hook = lambda: hook
        mod.set_axon_ntff_profile_hook = lambda h: None
        sys.modules["antenv.axon_hooks"] = mod
        antenv.axon_hooks = mod
        from concourse import bass_utils as _bu
        _bu.upload_artifacts = lambda tmpdir: ""
        return True
    except Exception:
        return False


def _run(nc, in_maps, cores, trace, tag):
    if trace:
        try:
            r = run_bass_kernel_spmd(nc, in_maps, cores, trace=True)
            LAST_EXEC[tag] = r.exec_time_ns
            LAST_RESULTS[tag] = r
            return r
        except Exception as e:
            print(f"trace run failed ({e!r}); retrying without trace")
    return run_bass_kernel_spmd(nc, in_maps, cores)


def kernel(node_features, adjacency_matrix, mask_father, neighbor_count,
           mask_hadamard, linear_w, linear_b, weight):
    nc1, nc2 = _get_ncs()
    trace = bool(int(os.environ.get("BASS_KERNEL_TRACE", "0"))) and _ensure_trace_hook()
    cores = list(range(M))
    bf = ml_dtypes.bfloat16
    f8 = ml_dtypes.float8_e4m3fn

    nf = np.ascontiguousarray(np.asarray(node_features, dtype=np.float32))
    A = np.ascontiguousarray(np.asarray(adjacency_matrix, dtype=np.float32))
    Ao = np.ascontiguousarray(np.asarray(mask_father, dtype=np.float32)[:, 0, :])
    S = np.ascontiguousarray(np.asarray(mask_hadamard, dtype=np.float32)[:, 0, :])
    ncnt = np.asarray(neighbor_count, dtype=np.float32)
    lw = np.asarray(linear_w, dtype=np.float32)
    lb = np.asarray(linear_b, dtype=np.float32)
    W = np.ascontiguousarray(np.asarray(weight, dtype=np.float32))

    # ---- launch A: wf rows ----
    lwT = np.ascontiguousarray(lw.T)                       # [F_RAW, IN_F]
    bias = np.ascontiguousarray(lb.reshape(IN_F // 128, 128).T)  # [128, 8]
    wbq = np.ascontiguousarray(
        W.astype(bf).reshape(NFB, 128, 8, 512).transpose(2, 1, 0, 3)
        .reshape(IN_F, C))
    in1 = []
    for m in range(M):
        sl = slice(m * JB, (m + 1) * JB)
        in1.append({
            "lwT": lwT,
            "nfT": np.ascontiguousarray(nf[sl, :].T),
            "bias": bias,
            "wbq": wbq,
        })
    r1 = _run(nc1, in1, cores, trace, "neffA")
    wfd = np.concatenate([r1.results[m]["wf_rows"] for m in range(M)], axis=0)
    wf32 = wfd.astype(np.float32)

    # ---- host reshaping (no compute beyond casts/elementwise staging) ----
    inv2 = (1.0 / np.square(ncnt.astype(np.float64)))[:, 0].astype(np.float32)
    a8q = np.ascontiguousarray(
        A.astype(f8).reshape(NKB, 128, 4, 1024).transpose(2, 1, 0, 3)
        .reshape(4 * 128, NKB * 1024))
    wfq = np.ascontiguousarray(
        wfd.reshape(NIB, 128, 8, 512).transpose(2, 1, 0, 3)
        .reshape(8 * 128, NIB * 512))
    in2 = []
    for m in range(M):
        sl = slice(m * JB, (m + 1) * JB)
        in2.append({
            "a8q": a8q,
            "acq": np.ascontiguousarray(
                Ao[:, sl].astype(f8).reshape(NKB, 128, JB)
                .transpose(1, 0, 2).reshape(128, NKB * JB)),
            "scq": np.ascontiguousarray(
                S[:, sl].astype(bf).reshape(NIB, 128, JB)
                .transpose(1, 0, 2).reshape(128, NIB * JB)),
            "wfq": wfq,
            "ws2": np.ascontiguousarray(wf32[sl, :] * inv2[None, :]),
        })
    r2 = _run(nc2, in2, cores, trace, "neffB")

    out = np.empty((C, N), dtype=np.float32)
    for m in range(M):
        out[:, m * JB:(m + 1) * JB] = r2.results[m]["ob"].T
    return out


# revision 21
# speedup vs baseline: 1.9957x; 1.0330x over previous
"""Distributed Bass kernel for nn_Interaction_GraphConvolution.

Math (reference):
    x  = node_features @ linear_w.T + linear_b          [N, IN_F]
    wf = x @ weight                                     [N, C]
    G  = mask_father[:,0,:].T @ adjacency               [N, N]
    P  = G * mask_hadamard[:,0,:].T                     [N, N]
    out[c, j] = wf[j,c] * (P @ wf)[j,c] / neighbor_count[c]^2

Sharding: node dim j split across 8 cores, 512 each (J_m).
Two SPMD launches:
  NEFF-A: core m computes wf rows J_m (Linear in f32r, projection GEMM in
    bf16), written out in bf16.
  NEFF-B: core m computes PT[:, J_m] = G[J_m,:].T * S[:, J_m] on-device
    (fp8 DoubleRow GEMM: adjacency 0/1 entries are exact in fp8; matmul
    oriented stationary=A-column-tiles / moving=mask_father-columns so
    the psum lands directly in [i, j] stationary orientation for the O
    phase - no transpose anywhere), then PS rows J_m via stationary-PT /
    moving-wf bf16 matmuls (psum in [j, c] orientation), with a fused
    epilogue multiply by the host-prescaled wf[J_m,:]*inv(ncnt^2).
Host between launches only reshapes/casts (gather wf, slice S columns,
fold inv2 into the elementwise factor); final output is the per-core
[j, c] block transposed into out[:, J_m].
Startup latency hiding: inputs land via few large strided DMA triggers
(critical-path tensors first, bulk tensors on a second queue), and a
dummy-matmul warmup burst keeps the PE HAM clock-gate at full rate
while the first inputs stream in.
Measured end-to-end max rel err ~4e-3 vs 2e-2 tolerance.
"""

import os
import sys

sys.path.insert(0, "/opt/trn_rl_repo")

import numpy as np
import ml_dtypes

from concourse import bass, bacc, mybir, tile
from concourse.bass_utils import run_bass_kernel_spmd

F32 = mybir.dt.float32
F32R = mybir.dt.float32r
BF16 = mybir.dt.bfloat16
F8 = mybir.dt.float8e4

N = 4096       # nodes (== out channels C)
F_RAW = 512    # raw feature dim
IN_F = 1024    # hidden dim
C = 4096       # out channels
M = 8          # cores
JB = N // M    # 512 nodes per core

NKB = N // 128   # 32 k-blocks
NIB = N // 128   # 32 i-blocks
NJB = JB // 128  # 4 j-blocks
NFB = IN_F // 128  # 8 f-blocks
NRB = F_RAW // 128  # 4 r-blocks

LAST_EXEC = {}
LAST_RESULTS = {}


def _warmup(nc, tc, n_mm=40):
    """Dummy matmul burst: keeps the PE busy (HAM stays at full clock)
    while the first real inputs stream in from HBM."""
    with tc.tile_pool(name="warm", bufs=1) as wp, \
         tc.tile_pool(name="pswarm", bufs=1, space=bass.MemorySpace.PSUM) as pwp:
        wtile = wp.tile([128, 512], BF16)
        nc.gpsimd.memset(wtile[:], 1.0)
        pwarm = pwp.tile([128, 512], F32, tag="pwarm")
        for _ in range(n_mm):
            nc.tensor.matmul(pwarm[:], wtile[:, 0:128], wtile[:],
                             start=True, stop=True)


def _build_neffA():
    """Per core m: wf rows J_m.

    Inputs: lwT [F_RAW, IN_F] f32r, nfT [F_RAW, JB] f32r, bias [128, 8] f32,
    wb [IN_F, C] bf16. Output: wf_rows [JB, C] bf16.
    """
    nc = bacc.Bacc()
    lwT_d = nc.dram_tensor("lwT", [F_RAW, IN_F], F32R, kind="ExternalInput")
    nfT_d = nc.dram_tensor("nfT", [F_RAW, JB], F32R, kind="ExternalInput")
    b_d = nc.dram_tensor("bias", [128, NFB], F32, kind="ExternalInput")
    # wbq[ch*128+p, fb*512+j] = W[fb*128+p, ch*512+j]: host-swizzled so each
    # 512-column chunk loads as one fully-contiguous DMA
    wb_d = nc.dram_tensor("wbq", [IN_F, C], BF16, kind="ExternalInput")
    wf_d = nc.dram_tensor("wf_rows", [JB, C], BF16, kind="ExternalOutput")

    with tile.TileContext(nc) as tc:
        _warmup(nc, tc)
        with tc.tile_pool(name="big", bufs=1) as bigp, \
             tc.tile_pool(name="psw", bufs=6, space=bass.MemorySpace.PSUM) as pswp:
            # psw is allocated before psx so the two never share a PSUM bank:
            # phase W's first matmuls can then overlap phase X's tail.
            xt_t = bigp.tile([128, NFB, JB], BF16)

            # ---- phase X: xT = lw @ nf[J_m].T + b ----
            with tc.tile_pool(name="xin", bufs=1) as xinp, \
                 tc.tile_pool(name="psx", bufs=2, space=bass.MemorySpace.PSUM) as psxp:
                lwT_t = xinp.tile([128, NRB, IN_F], F32R)
                nc.sync.dma_start(
                    lwT_t[:], lwT_d[:].rearrange("(rb p) f -> p rb f", p=128))
                nfT_t = xinp.tile([128, NRB, JB], F32R)
                nc.sync.dma_start(
                    nfT_t[:], nfT_d[:].rearrange("(rb p) j -> p rb j", p=128))
                b_t = xinp.tile([128, NFB], F32)
                nc.sync.dma_start(b_t[:], b_d[:])
                for fb in range(NFB):
                    psx = psxp.tile([128, JB], F32, tag="psx")
                    for rb in range(NRB):
                        nc.tensor.matmul(
                            psx[:],
                            lwT_t[:, rb, fb * 128:(fb + 1) * 128],
                            nfT_t[:, rb, :],
                            start=(rb == 0), stop=(rb == NRB - 1))
                    nc.scalar.activation(
                        xt_t[:, fb, :], psx[:],
                        mybir.ActivationFunctionType.Identity,
                        bias=b_t[:, fb:fb + 1], scale=1.0)

            # ---- phase W: wf[J_m] = xT.T @ W (bf16) ----
            with tc.tile_pool(name="wst", bufs=2) as wstp, \
                 tc.tile_pool(name="wout", bufs=4) as woutp:
                for ch in range(C // 512):
                    wb_t = wstp.tile([128, NFB, 512], BF16, tag="wb")
                    nc.sync.dma_start(
                        wb_t[:],
                        wb_d[ch * 128:(ch + 1) * 128, :].rearrange(
                            "p (fb c) -> p fb c", fb=NFB))
                    pw = [pswp.tile([128, 512], F32, tag="pw", name=f"pw{_i}")
                          for _i in range(NJB)]
                    for fb in range(NFB):
                        for jb in range(NJB):
                            nc.tensor.matmul(
                                pw[jb][:],
                                xt_t[:, fb, jb * 128:(jb + 1) * 128],
                                wb_t[:, fb, :],
                                start=(fb == 0), stop=(fb == NFB - 1))
                    for jb in range(NJB):
                        o_sb = woutp.tile([128, 512], BF16, tag="o_sb")
                        nc.vector.tensor_copy(o_sb[:], pw[jb][:])
                        nc.sync.dma_start(
                            wf_d[jb * 128:(jb + 1) * 128, ch * 512:(ch + 1) * 512],
                            o_sb[:])
    nc.finalize()
    return nc


def _build_neffB():
    """Per core m: G rows J_m -> PT, PS rows J_m, fused epilogue.

    Inputs: a8 [N, N] fp8 (adjacency), ac8 [N, JB] fp8 (mask_father cols),
    sc [N, JB] bf16 (S cols J_m), wfd [N, C] bf16 (full wf),
    ws2 [JB, C] f32 (wf[J_m,:] * inv2[c]).
    Output: ob [JB, C] f32 with ob[j, c] = out[c, J_m[j]].
    """
    nc = bacc.Bacc()
    # All bulk inputs are host-swizzled so every DMA reads fully-contiguous
    # lines (strided sub-2KB lines halve effective HBM bandwidth):
    #   a8q [iq*128+p, kb*1024+i'] = A[kb*128+p, iq*1024+i']
    #   acq [p, kb*512+j]          = mf[kb*128+p, J_m[j]]
    #   scq [p, ib*512+j]          = S[ib*128+p, J_m[j]]
    #   wfq [ce*128+p, ib*512+c']  = wf[ib*128+p, ce*512+c']
    a_d = nc.dram_tensor("a8q", [4 * 128, NKB * 1024], F8, kind="ExternalInput")
    ac_d = nc.dram_tensor("acq", [128, NKB * JB], F8, kind="ExternalInput")
    sc_d = nc.dram_tensor("scq", [128, NIB * JB], BF16, kind="ExternalInput")
    wfd_d = nc.dram_tensor("wfq", [8 * 128, NIB * 512], BF16, kind="ExternalInput")
    ws2_d = nc.dram_tensor("ws2", [JB, C], F32, kind="ExternalInput")
    ob_d = nc.dram_tensor("ob", [JB, C], F32, kind="ExternalOutput")

    NIQ = 4          # i-quarters for the A stream (G stationary)
    IQ = NIB // NIQ  # 8 i-blocks per quarter
    NCE = 8          # c-eighths for the wf stream (O moving)
    CE = C // NCE    # 512
    SCH = 4          # sc DMA chunk: 4 i-blocks per trigger

    with tile.TileContext(nc) as tc:
        _warmup(nc, tc)
        with tc.tile_pool(name="pt", bufs=1) as ptp, \
             tc.tile_pool(name="wq", bufs=2) as wqp:
            pt_t = ptp.tile([128, NIB, JB], BF16)
            # wfd streams on the scalar HWDGE queue while G runs on data
            # from the sync queue; the wq pool is opened before the G pools
            # so the transfers are not serialized behind G's SBUF reuse.
            # The first two prefetches are issued late in phase G so they
            # don't compete with G's own input stream at launch.
            wq_tiles = [wqp.tile([128, NIB, CE], BF16, tag="wq", name=f"wq{_i}")
                        for _i in range(NCE)]

            def _wq_fetch(ce, eng=None):
                # The first two prefetches ride the sync queue *behind* G's
                # own input stream (engine queues run triggers as soon as
                # they reach the head — code position alone delays nothing).
                (eng or nc.scalar).dma_start(
                    wq_tiles[ce][:],
                    wfd_d[ce * 128:(ce + 1) * 128, :].rearrange(
                        "p (ib c) -> p ib c", ib=NIB))

            # ---- phase G: pt[i, j] = (A^T @ mf_cols)[i, j] * S[i, J_m[j]] ----
            with tc.tile_pool(name="acm", bufs=1) as acmp, \
                 tc.tile_pool(name="aq", bufs=2) as aqp, \
                 tc.tile_pool(name="scp", bufs=2) as scp, \
                 tc.tile_pool(name="psg", bufs=4, space=bass.MemorySpace.PSUM) as psgp:
                HKC = NKB // 2 * 1024   # half the columns of one a8q row-block
                ac_t = acmp.tile([128, NKB, JB], F8)
                for h in range(2):
                    nc.sync.dma_start(
                        ac_t[:, h * (NKB // 2):(h + 1) * (NKB // 2), :],
                        ac_d[:, h * (NKB // 2) * JB:(h + 1) * (NKB // 2) * JB]
                        .rearrange("p (kb j) -> p kb j", kb=NKB // 2))
                for iq in range(NIQ):
                    aq_t = aqp.tile([128, NKB, IQ * 128], F8, tag="aq")
                    for h in range(2):
                        nc.sync.dma_start(
                            aq_t[:, h * (NKB // 2):(h + 1) * (NKB // 2), :],
                            a_d[iq * 128:(iq + 1) * 128, h * HKC:(h + 1) * HKC]
                            .rearrange("p (kb i) -> p kb i", kb=NKB // 2))
                    if iq == 1:
                        _wq_fetch(0, nc.sync)
                    elif iq == 2:
                        _wq_fetch(1, nc.sync)
                    for ib8 in range(IQ):
                        ib = iq * IQ + ib8
                        if ib % SCH == 0:
                            sc_t = scp.tile([128, SCH, JB], BF16, tag="sc")
                            nc.sync.dma_start(
                                sc_t[:],
                                sc_d[:, ib * JB:(ib + SCH) * JB].rearrange(
                                    "p (b j) -> p b j", b=SCH))
                        pg = psgp.tile([128, JB], F32, tag="pg")
                        for kt in range(NKB // 2):
                            nc.tensor.matmul(
                                pg[:],
                                aq_t[:, 2 * kt:2 * kt + 2, ib8 * 128:(ib8 + 1) * 128],
                                ac_t[:, 2 * kt:2 * kt + 2, :],
                                start=(kt == 0), stop=(kt == NKB // 2 - 1),
                                perf_mode=mybir.MatmulPerfMode.DoubleRow)
                        nc.vector.tensor_mul(
                            pt_t[:, ib, :], pg[:], sc_t[:, ib % SCH, :])

            # ---- phase O: ob[j, c] = (pt.T @ wf)[j, c] * ws2[j, c] ----
            with tc.tile_pool(name="psb", bufs=6, space=bass.MemorySpace.PSUM) as psbp, \
                 tc.tile_pool(name="w2p", bufs=3) as w2p, \
                 tc.tile_pool(name="eout", bufs=3) as eoutp:
                for ce in range(NCE):
                    wq_t = wq_tiles[ce]
                    if ce >= 2:
                        _wq_fetch(ce)
                    for jb in range(NJB):
                        po = psbp.tile([128, CE], F32, tag="po")
                        for ib in range(NIB):
                            nc.tensor.matmul(
                                po[:],
                                pt_t[:, ib, jb * 128:(jb + 1) * 128],
                                wq_t[:, ib, :],
                                start=(ib == 0), stop=(ib == NIB - 1))
                        w2_t = w2p.tile([128, CE], F32, tag="w2")
                        nc.sync.dma_start(
                            w2_t[:],
                            ws2_d[jb * 128:(jb + 1) * 128, ce * CE:(ce + 1) * CE])
                        o_sb = eoutp.tile([128, CE], F32, tag="o_sb")
                        nc.vector.tensor_mul(o_sb[:], po[:], w2_t[:])
                        nc.sync.dma_start(
                            ob_d[jb * 128:(jb + 1) * 128, ce * CE:(ce + 1) * CE],
                            o_sb[:])
    nc.finalize()
    return nc


_NC1 = None
_NC2 = None


def _get_ncs():
    global _NC1, _NC2
    if _NC1 is None:
        _NC1 = _build_neffA()
        _NC2 = _build_neffB()
    return _NC1, _NC2


def _ensure_trace_hook():
    """Best-effort NTFF profiling shim (test harness only; grading runs
    without tracing). The agent image's antenv lacks axon_hooks, but the
    axon boot package exposes the ctypes equivalent."""
    try:
        from antenv.axon_hooks import get_axon_ntff_profile_hook
        return get_axon_ntff_profile_hook() is not None
    except ImportError:
        pass
    try:
        import types
        if "/root/.axon_site" not in sys.path:
            sys.path.insert(0, "/root/.axon_site")
        from trn_agent_boot.trn_boot import _ntff_profile_via_ctypes
        hook = _ntff_profile_via_ctypes("/opt/axon/libaxon_pjrt.so")
        if hook is None:
            return False
        import antenv
        mod = types.ModuleType("antenv.axon_hooks")
        mod.get_axon_ntff_profile_hook = lambda: hook
        mod.set_axon_ntff_profile_hook = lambda h: None
        sys.modules["antenv.axon_hooks"] = mod
        antenv.axon_hooks = mod
        from concourse import bass_utils as _bu
        _bu.upload_artifacts = lambda tmpdir: ""
        return True
    except Exception:
        return False


def _run(nc, in_maps, cores, trace, tag):
    if trace:
        try:
            r = run_bass_kernel_spmd(nc, in_maps, cores, trace=True)
            LAST_EXEC[tag] = r.exec_time_ns
            LAST_RESULTS[tag] = r
            return r
        except Exception as e:
            print(f"trace run failed ({e!r}); retrying without trace")
    return run_bass_kernel_spmd(nc, in_maps, cores)


def kernel(node_features, adjacency_matrix, mask_father, neighbor_count,
           mask_hadamard, linear_w, linear_b, weight):
    nc1, nc2 = _get_ncs()
    trace = bool(int(os.environ.get("BASS_KERNEL_TRACE", "0"))) and _ensure_trace_hook()
    cores = list(range(M))
    bf = ml_dtypes.bfloat16
    f8 = ml_dtypes.float8_e4m3fn

    nf = np.ascontiguousarray(np.asarray(node_features, dtype=np.float32))
    A = np.ascontiguousarray(np.asarray(adjacency_matrix, dtype=np.float32))
    Ao = np.ascontiguousarray(np.asarray(mask_father, dtype=np.float32)[:, 0, :])
    S = np.ascontiguousarray(np.asarray(mask_hadamard, dtype=np.float32)[:, 0, :])
    ncnt = np.asarray(neighbor_count, dtype=np.float32)
    lw = np.asarray(linear_w, dtype=np.float32)
    lb = np.asarray(linear_b, dtype=np.float32)
    W = np.ascontiguousarray(np.asarray(weight, dtype=np.float32))

    # ---- launch A: wf rows ----
    lwT = np.ascontiguousarray(lw.T)                       # [F_RAW, IN_F]
    bias = np.ascontiguousarray(lb.reshape(IN_F // 128, 128).T)  # [128, 8]
    wbq = np.ascontiguousarray(
        W.astype(bf).reshape(NFB, 128, 8, 512).transpose(2, 1, 0, 3)
        .reshape(IN_F, C))
    in1 = []
    for m in range(M):
        sl = slice(m * JB, (m + 1) * JB)
        in1.append({
            "lwT": lwT,
            "nfT": np.ascontiguousarray(nf[sl, :].T),
            "bias": bias,
            "wbq": wbq,
        })
    r1 = _run(nc1, in1, cores, trace, "neffA")
    wfd = np.concatenate([r1.results[m]["wf_rows"] for m in range(M)], axis=0)
    wf32 = wfd.astype(np.float32)

    # ---- host reshaping (no compute beyond casts/elementwise staging) ----
    inv2 = (1.0 / np.square(ncnt.astype(np.float64)))[:, 0].astype(np.float32)
    a8q = np.ascontiguousarray(
        A.astype(f8).reshape(NKB, 128, 4, 1024).transpose(2, 1, 0, 3)
        .reshape(4 * 128, NKB * 1024))
    wfq = np.ascontiguousarray(
        wfd.reshape(NIB, 128, 8, 512).transpose(2, 1, 0, 3)
        .reshape(8 * 128, NIB * 512))
    in2 = []
    for m in range(M):
        sl = slice(m * JB, (m + 1) * JB)
        in2.append({
            "a8q": a8q,
            "acq": np.ascontiguousarray(
                Ao[:, sl].astype(f8).reshape(NKB, 128, JB)
                .transpose(1, 0, 2).reshape(128, NKB * JB)),
            "scq": np.ascontiguousarray(
                S[:, sl].astype(bf).reshape(NIB, 128, JB)
                .transpose(1, 0, 2).reshape(128, NIB * JB)),
            "wfq": wfq,
            "ws2": np.ascontiguousarray(wf32[sl, :] * inv2[None, :]),
        })
    r2 = _run(nc2, in2, cores, trace, "neffB")

    out = np.empty((C, N), dtype=np.float32)
    for m in range(M):
        out[:, m * JB:(m + 1) * JB] = r2.results[m]["ob"].T
    return out
